# revision 1
# baseline (speedup 1.0000x reference)
"""Trainium2 Bass kernel for nn_Attention_9096740733536 (sparse_attention).

Sharding: data-parallel over the QB (task) dim across 8 cores (2 tasks/core),
one mid-kernel AllReduce of [feat_corr partials | q_global | k_global] sums.
The attention math is algebraically collapsed: mixed scores are linear (no
softmax), so
  out[h,q] = alpha_h*(Fq/qn) @ ((Fk/kn)^T @ Fv) + ww_h*q_ratio (x) (k_ratio^T Fv)
with 128x128 inner matrices instead of 512x512 score matrices, and layernorm
is folded into the input projection via rank-1 PSUM augmentation.
"""
import numpy as np
from contextlib import ExitStack

import concourse.bass as bass
import concourse.tile as tile
from concourse import bacc, mybir
from concourse import bass_utils
from concourse._compat import with_exitstack

F32 = mybir.dt.float32
F32R = mybir.dt.float32r
AF = mybir.ActivationFunctionType
ALU = mybir.AluOpType
AX = mybir.AxisListType

H, D, DIM = 8, 128, 1024
QB, N = 16, 512
N_CORES = 8
T = QB * N // N_CORES          # 1024 tokens per core
NT = T // 128                  # 8 token tiles per core
NTASK = T // N                 # 2 tasks per core
LN_EPS = 1e-5
TOK_ALL = float(QB * N)


@with_exitstack
def attn_kernel(ctx: ExitStack, tc: tile.TileContext, outs, ins, n_cores=N_CORES):
    nc = tc.nc
    y = outs[0]
    (xn_q, xn_k, xn_v, xT_q, xT_k, xT_v, Wp_d, WoT_d, negu_d, vrow_d,
     bout_d, ones_d, ident_d, mask_d, wp1T_d, wp2T_d, b1_d, gbc_d, bbc_d,
     b2bc_d) = ins

    consts = ctx.enter_context(tc.tile_pool(name="consts", bufs=1))
    fpool = ctx.enter_context(tc.tile_pool(name="fpool", bufs=1))
    stat1 = ctx.enter_context(tc.tile_pool(name="stat1", bufs=1))
    dram = ctx.enter_context(tc.tile_pool(name="dram", bufs=1, space="DRAM"))

    ps_proj = ctx.enter_context(tc.tile_pool(name="ps_proj", bufs=3, space="PSUM"))
    ps_fc = ctx.enter_context(tc.tile_pool(name="ps_fc", bufs=2, space="PSUM"))
    ps_gk = ctx.enter_context(tc.tile_pool(name="ps_gk", bufs=1, space="PSUM"))
    ps_o1 = ctx.enter_context(tc.tile_pool(name="ps_o1", bufs=1, space="PSUM"))
    ps_small = ctx.enter_context(tc.tile_pool(name="ps_small", bufs=1, space="PSUM"))

    # ---- small constants (long-lived) ----
    ident = consts.tile([128, 128], F32)
    nc.sync.dma_start(ident[:], ident_d[:])
    bout = consts.tile([1, DIM], F32R)
    nc.sync.dma_start(bout[:], bout_d[:].bitcast(F32R))
    onesr = consts.tile([1, 128], F32R)
    nc.sync.dma_start(onesr[:], ones_d[0:1, :].bitcast(F32R))
    ones = consts.tile([128, 8], F32)
    nc.sync.dma_start(ones[:], ones_d[:, 0:8])
    mask_nd = consts.tile([128, H * 128], F32)
    nc.scalar.dma_start(mask_nd[:], mask_d[:])
    wp1T = consts.tile([128, 256], F32)
    nc.scalar.dma_start(wp1T[:], wp1T_d[:])
    wp2T = consts.tile([128, 3], F32)
    nc.scalar.dma_start(wp2T[:], wp2T_d[:])
    b1row = consts.tile([1, 128], F32)
    nc.scalar.dma_start(b1row[:], b1_d[:])
    ones8 = consts.tile([1, 8], F32)
    nc.sync.dma_start(ones8[:], ones_d[0:1, 0:8])
    gbc = consts.tile([8, 128], F32)
    nc.scalar.dma_start(gbc[:], gbc_d[:])
    bbc = consts.tile([8, 128], F32)
    nc.scalar.dma_start(bbc[:], bbc_d[:])
    b2bc = consts.tile([8, 3], F32)
    nc.scalar.dma_start(b2bc[:], b2bc_d[:])
    eps = consts.tile([128, 1], F32)
    nc.vector.memset(eps[:], LN_EPS)

    # ---- persistent F tensors: [128 tok, t*1024 + h*128 + d] ----
    Fq = fpool.tile([128, NT * DIM], F32)
    Fk = fpool.tile([128, NT * DIM], F32)
    Fv = fpool.tile([128, NT * DIM], F32)
    sq_scr = stat1.tile([128, DIM], F32)     # ACT square scratch (write-only)

    xns = [xn_q, xn_k, xn_v]
    xTs = [xT_q, xT_k, xT_v]
    Fs = [Fq, Fk, Fv]

    # ======== Phase 1: folded-LN projection (scoped pools) ========
    with tc.tile_pool(name="ph1", bufs=1) as ph1, \
         tc.tile_pool(name="xpool", bufs=3) as xpool, \
         tc.tile_pool(name="spool", bufs=3) as spool:
        Wp = ph1.tile([128, 8 * DIM], F32R)
        for s in range(8):
            nc.gpsimd.dma_start(Wp[:, s * DIM:(s + 1) * DIM],
                                Wp_d[:, s * DIM:(s + 1) * DIM].bitcast(F32R))
        negu = ph1.tile([1, DIM], F32R)
        nc.sync.dma_start(negu[:], negu_d[:].bitcast(F32R))
        vrow = ph1.tile([1, DIM], F32R)
        nc.sync.dma_start(vrow[:], vrow_d[:].bitcast(F32R))
        for t in range(NT):
            st = spool.tile([128, 12], F32, tag="st")
            bn6 = spool.tile([128, 36], F32, tag="bn6")
            rsig = spool.tile([128, 3], F32, tag="rsig")
            for i in range(3):
                xn = xpool.tile([128, DIM], F32, tag="xn")
                nc.sync.dma_start(xn[:], xns[i][t * 128:(t + 1) * 128, :])
                nc.vector.bn_stats(bn6[:, i * 12:i * 12 + 6], xn[:, 0:512])
                nc.vector.bn_stats(bn6[:, i * 12 + 6:i * 12 + 12],
                                   xn[:, 512:1024])
                # (mean, var) pair -> st cols (6+i, 9+i via sqrt)
                nc.vector.bn_aggr(st[:, 2 * i:2 * i + 2],
                                  bn6[:, i * 12:i * 12 + 12])
            # st cols 0,2,4 = mu ; 1,3,5 = var
            nc.vector.tensor_copy(st[:, 6:9], st[:, 0:6:2])
            nc.scalar.activation(st[:, 9:12], st[:, 1:6:2], AF.Sqrt,
                                 bias=eps[:])
            nc.vector.reciprocal(rsig[:], st[:, 9:12])
            # transpose [mu|sig] (cols 6..11) -> rows [6, 128] -> flat [1, 768]
            trp = ps_small.tile([6, 128], F32, tag="sm")
            nc.tensor.transpose(trp[:], st[:, 6:12], ident[:])
            rows6 = spool.tile([6, 128], F32R, tag="rows6")
            nc.scalar.copy(rows6[:], trp[:])
            rows = spool.tile([1, 768], F32R, tag="rows")
            nc.scalar.dma_start(rows[:], rows6[:])
            for i in range(3):
                xT_t = xpool.tile([128, DIM], F32R, tag="xT")
                nc.sync.dma_start(xT_t[:],
                                  xTs[i][:, t * DIM:(t + 1) * DIM].bitcast(F32R))
                for half in range(2):
                    o = half * 512
                    acc = ps_proj.tile([128, 512], F32, tag="proj")
                    for s in range(8):
                        nc.tensor.matmul(
                            acc[:], xT_t[:, s * 128:(s + 1) * 128],
                            Wp[:, s * DIM + o: s * DIM + o + 512],
                            start=(s == 0), stop=False)
                    nc.tensor.matmul(acc[:], rows[:, i * 128:(i + 1) * 128],
                                     negu[:, o:o + 512], start=False, stop=False)
                    nc.tensor.matmul(acc[:], rows[:, (3 + i) * 128:(4 + i) * 128],
                                     vrow[:, o:o + 512], start=False, stop=True)
                    dst = Fs[i][:, t * DIM + o: t * DIM + o + 512]
                    if (i + half) % 2 == 0:
                        nc.scalar.mul(dst, acc[:], rsig[:, i:i + 1])
                    else:
                        nc.vector.tensor_scalar_mul(dst, acc[:],
                                                    rsig[:, i:i + 1])

    # ======== Phase 2: F stats, feat_corr partials, q/k globals ========
    late = ctx.enter_context(tc.tile_pool(name="late", bufs=1))
    WoT = late.tile([128, 8 * DIM], F32R)
    nc.gpsimd.dma_start(WoT[:], WoT_d[:].bitcast(F32R))

    qss = stat1.tile([128, 64], F32)   # col t*8+h : sumsq over d of Fq
    qsm = stat1.tile([128, 64], F32)   # sums over d
    kss = stat1.tile([128, 64], F32)
    ksm = stat1.tile([128, 64], F32)
    qmean = stat1.tile([128, 64], F32)
    qninv = stat1.tile([128, 64], F32)
    kninv = stat1.tile([128, 64], F32)
    kn = stat1.tile([128, 64], F32)
    qr = stat1.tile([128, 64], F32)
    kr = stat1.tile([128, 64], F32)
    rscr = stat1.tile([128, 96], F32)  # ratio-chain scratch (3x32 per half)

    def derived(ss, sm, ninv, ratio, s, n_out=None):
        # ninv = 1/sqrt(ss); var = ss/127 - sm^2/(128*127)
        # ratio = 2*min(var,1)/(var+1)
        w = s.stop - s.start
        if n_out is not None:
            nc.scalar.activation(n_out[:, s], ss[:, s], AF.Sqrt)
            nc.vector.reciprocal(ninv[:, s], n_out[:, s])
        else:
            nc.scalar.activation(ninv[:, s], ss[:, s], AF.Sqrt)
            nc.vector.reciprocal(ninv[:, s], ninv[:, s])
        t1 = rscr[:, 0:w]
        nc.vector.tensor_tensor(t1, sm[:, s], sm[:, s], op=ALU.mult)
        nc.vector.tensor_scalar_mul(t1, t1, 1.0 / (D * (D - 1)))
        t2 = rscr[:, w:2 * w]
        nc.vector.tensor_scalar_mul(t2, ss[:, s], 1.0 / (D - 1))
        var = rscr[:, 2 * w:3 * w]
        nc.vector.tensor_tensor(var, t2, t1, op=ALU.subtract)
        nc.vector.tensor_scalar(t1, var, 1.0, 2.0, ALU.min, ALU.mult)
        nc.vector.tensor_scalar_add(t2, var, 1.0)
        nc.vector.reciprocal(t2, t2)
        nc.vector.tensor_tensor(ratio[:, s], t1, t2, op=ALU.mult)

    for jh in range(NTASK):
        for t in range(4 * jh, 4 * jh + 4):
            nc.vector.reduce_sum(
                qsm[:, t * 8:(t + 1) * 8],
                Fq[:, t * DIM:(t + 1) * DIM].rearrange("p (h d) -> p h d", h=8),
                axis=AX.X)
            nc.vector.reduce_sum(
                ksm[:, t * 8:(t + 1) * 8],
                Fk[:, t * DIM:(t + 1) * DIM].rearrange("p (h d) -> p h d", h=8),
                axis=AX.X)
            for h in range(H):
                sl = slice(t * DIM + h * 128, t * DIM + h * 128 + 128)
                nc.scalar.activation(sq_scr[:, 0:128], Fq[:, sl], AF.Square,
                                     accum_out=qss[:, t * 8 + h:t * 8 + h + 1])
                nc.scalar.activation(sq_scr[:, 128:256], Fk[:, sl], AF.Square,
                                     accum_out=kss[:, t * 8 + h:t * 8 + h + 1])
        s = slice(jh * 32, jh * 32 + 32)
        # NOTE: qmean holds NEGATED means (used as ACT bias for centering)
        nc.vector.tensor_scalar_mul(qmean[:, s], qsm[:, s], -1.0 / D)
        derived(qss, qsm, qninv, qr, s)
        derived(kss, ksm, kninv, kr, s, n_out=kn)
        # absorb kn into k_ratio: mv uses scaled Fv, so kr must carry kn back
        nc.vector.tensor_tensor(kr[:, s], kr[:, s], kn[:, s], op=ALU.mult)
        # scale Fv in place by 1/kn (only consumer is the M/mv stage)
        for t in range(4 * jh, 4 * jh + 4):
            for h in range(H):
                sl = slice(t * DIM + h * 128, t * DIM + h * 128 + 128)
                nc.vector.tensor_scalar(Fv[:, sl], Fv[:, sl],
                                        kninv[:, t * 8 + h:t * 8 + h + 1],
                                        None, ALU.mult)

    # ======== Phase 4a: allreduce-independent M/mv stage ========
    # M = Fk^T @ (Fv/kn) and mv = (kr*kn)^T @ (Fv/kn) per (head, task),
    # evicted UNSCALED (alpha/ww applied post-allreduce). Placed BEFORE the
    # feat_corr stage so the in-order PE stream overlaps the phase-1 tail.
    attn = ctx.enter_context(tc.tile_pool(name="attn", bufs=1))
    mm_raw = {}
    mv_raw = {}
    for j in range(NTASK):
        for h in range(H):
            mm_ps = ps_fc.tile([128, 128], F32, tag="fc128", name="mm_ps")
            mv_ps = ps_small.tile([1, 128], F32, tag="sm", name="mv_ps")
            for ti in range(4):
                t = 4 * j + ti
                sl = slice(t * DIM + h * 128, t * DIM + h * 128 + 128)
                nc.tensor.matmul(mm_ps[:], Fk[:, sl], Fv[:, sl],
                                 start=(ti == 0), stop=(ti == 3))
                nc.tensor.matmul(mv_ps[:], kr[:, t * 8 + h:t * 8 + h + 1],
                                 Fv[:, sl], start=(ti == 0), stop=(ti == 3))
            mm = attn.tile([128, 128], F32R, tag=f"mm{h}{j}", name="mm")
            nc.scalar.copy(mm[:], mm_ps[:])
            mv = attn.tile([1, 128], F32R, tag=f"mv{h}{j}", name="mv")
            nc.scalar.copy(mv[:], mv_ps[:])
            mm_raw[(h, j)] = mm
            mv_raw[(h, j)] = mv

    # feat_corr partials (per head) + q/k global sums (single PSUM group)
    # t-outer emission so no engine stream blocks on the last proj tile.
    ar_in = dram.tile([128, H * 128 + 16], F32)
    ar_out = dram.tile([128, H * 128 + 16], F32)
    gk_ps = ps_gk.tile([128, 16], F32, tag="gk")
    with tc.tile_pool(name="ph2", bufs=2) as ph2, \
         tc.tile_pool(name="qcpool", bufs=64) as qcpool:
        qc_tiles = {}
        for t in range(NT):
            for h in range(H):
                sl = slice(t * DIM + h * 128, t * DIM + h * 128 + 128)
                qc = qcpool.tile([128, 128], mybir.dt.bfloat16, tag="qc",
                                 name="qc")
                nc.scalar.activation(qc[:], Fq[:, sl], AF.Identity,
                                     bias=qmean[:, t * 8 + h:t * 8 + h + 1])
                qc_tiles[(t, h)] = qc
                first = (h == 0 and t == 0)
                last = (h == H - 1 and t == NT - 1)
                nc.tensor.matmul(gk_ps[:, h:h + 1], Fq[:, sl], ones[:, 0:1],
                                 start=first, stop=last, skip_group_check=True)
                nc.tensor.matmul(gk_ps[:, 8 + h:9 + h], Fk[:, sl], ones[:, 0:1],
                                 start=False, stop=False, skip_group_check=True)
        for h in range(H):
            fc_ps = ps_fc.tile([128, 128], F32, tag="fc128", name="fc_ps")
            for t in range(NT):
                nc.tensor.matmul(fc_ps[:], qc_tiles[(t, h)][:],
                                 qc_tiles[(t, h)][:],
                                 start=(t == 0), stop=(t == NT - 1))
            fc_sb = ph2.tile([128, 128], F32, tag="fcsb", name="fc_sb")
            nc.vector.tensor_copy(fc_sb[:], fc_ps[:])
            nc.sync.dma_start(ar_in[:, h * 128:(h + 1) * 128], fc_sb[:])
        gk_sb = ph2.tile([128, 16], F32, tag="gksb", name="gk_sb")
        nc.scalar.copy(gk_sb[:], gk_ps[:])
        nc.sync.dma_start(ar_in[:, H * 128:H * 128 + 16], gk_sb[:])

    # in-place Fq <- Fq/qn (after feat_corr reads; gates only phase 4b)
    for h in range(H):
        for t in range(NT):
            sl = slice(t * DIM + h * 128, t * DIM + h * 128 + 128)
            c = slice(t * 8 + h, t * 8 + h + 1)
            nc.vector.tensor_scalar(Fq[:, sl], Fq[:, sl], qninv[:, c], None,
                                    ALU.mult)

    # ======== AllReduce ========
    if n_cores > 1:
        nc.gpsimd.collective_compute(
            "AllReduce", ALU.add,
            replica_groups=[list(range(n_cores))],
            ins=[ar_in.opt()], outs=[ar_out.opt()])
    else:  # single-core sim variant: allreduce over one core == copy
        nc.sync.dma_start(ar_out[:], ar_in[:])
    ar = late.tile([128, H * 128 + 16], F32)
    nc.sync.dma_start(ar[:], ar_out[:])
    arg = ar[:, H * 128:H * 128 + 16]

    # ======== Phase 3: decorr scale + weight predictor ========
    ssq = stat1.tile([128, 8], F32)
    msk = late.tile([128, H * 128], F32)
    nc.vector.tensor_tensor(msk[:], ar[:, 0:H * 128], mask_nd[:], op=ALU.mult)
    nc.scalar.activation(sq_scr[:, 0:H * 128], msk[:], AF.Square,
                         scale=1.0 / TOK_ALL)
    nc.vector.reduce_sum(ssq[:],
                         sq_scr[:, 0:H * 128].rearrange("p (h d) -> p h d", h=8),
                         axis=AX.X)
    ss_ps = ps_small.tile([8, 8], F32, tag="sm", name="ss_ps")
    nc.tensor.matmul(ss_ps[:], ssq[:], ones[:, 0:8], start=True, stop=True)
    dsc = stat1.tile([8, 8], F32)
    nc.scalar.activation(dsc[:, 0:1], ss_ps[0:8, 0:1], AF.Sqrt)
    nc.scalar.activation(dsc[:, 1:2], dsc[:, 0:1], AF.Exp, scale=-5.0 / (D * D))

    featsq = stat1.tile([128, 8], F32)
    nc.vector.tensor_scalar_mul(featsq[:], arg[:, 0:8], 1.0 / TOK_ALL)
    featsk = stat1.tile([128, 8], F32)
    nc.vector.tensor_scalar_mul(featsk[:], arg[:, 8:16], 1.0 / TOK_ALL)
    h1_ps = ps_small.tile([8, 128], F32, tag="sm", name="h1_ps")
    nc.tensor.matmul(h1_ps[:], featsq[:], wp1T[:, 0:128], start=True, stop=False)
    nc.tensor.matmul(h1_ps[:], featsk[:], wp1T[:, 128:256], start=False,
                     stop=False)
    nc.tensor.matmul(h1_ps[:], ones8[:], b1row[:], start=False, stop=True)
    h1 = stat1.tile([8, 128], F32)
    nc.scalar.copy(h1[:], h1_ps[:])
    w_mu = stat1.tile([8, 4], F32)
    nc.vector.reduce_sum(w_mu[:, 0:1], h1[:], axis=AX.X)
    nc.vector.tensor_scalar_mul(w_mu[:, 0:1], w_mu[:, 0:1], 1.0 / D)
    nc.scalar.activation(sq_scr[0:8, 0:128], h1[:], AF.Square,
                         accum_out=w_mu[:, 1:2])
    nc.vector.tensor_scalar_mul(w_mu[:, 1:2], w_mu[:, 1:2], 1.0 / D)
    nc.vector.tensor_tensor(w_mu[:, 2:3], w_mu[:, 0:1], w_mu[:, 0:1], op=ALU.mult)
    nc.vector.tensor_tensor(w_mu[:, 2:3], w_mu[:, 1:2], w_mu[:, 2:3],
                            op=ALU.subtract)
    nc.scalar.activation(w_mu[:, 3:4], w_mu[:, 2:3], AF.Sqrt, bias=eps[0:8, :])
    nc.vector.reciprocal(w_mu[:, 3:4], w_mu[:, 3:4])
    h1n = stat1.tile([8, 128], F32)
    nc.vector.tensor_scalar(h1n[:], h1[:], w_mu[:, 0:1], w_mu[:, 3:4],
                            ALU.subtract, ALU.mult)
    nc.vector.tensor_tensor(h1n[:], h1n[:], gbc[:], op=ALU.mult)
    nc.vector.tensor_tensor(h1n[:], h1n[:], bbc[:], op=ALU.add)
    nc.vector.tensor_scalar_max(h1n[:], h1n[:], 0.0)
    h1T_ps = ps_small.tile([128, 8], F32, tag="sm", name="h1T_ps")
    nc.tensor.transpose(h1T_ps[:], h1n[:], ident[0:8, 0:8])
    h1T = stat1.tile([128, 8], F32)
    nc.scalar.copy(h1T[:], h1T_ps[:])
    lg_ps = ps_small.tile([8, 3], F32, tag="sm", name="lg_ps")
    nc.tensor.matmul(lg_ps[:], h1T[:], wp2T[:], start=True, stop=True)
    lg = stat1.tile([8, 8], F32)
    nc.scalar.copy(lg[:, 0:3], lg_ps[:])
    nc.vector.tensor_tensor(lg[:, 0:3], lg[:, 0:3], b2bc[:], op=ALU.add)
    # logits are O(1): skip the (mathematically redundant) max-subtraction
    nc.scalar.activation(lg[:, 0:3], lg[:, 0:3], AF.Exp)
    nc.vector.reduce_sum(lg[:, 4:5], lg[:, 0:3], axis=AX.X)
    nc.vector.reciprocal(lg[:, 4:5], lg[:, 4:5])
    nc.vector.tensor_scalar(lg[:, 0:3], lg[:, 0:3], lg[:, 4:5], None, ALU.mult)
    # alpha = w0 + w1*dsc ; ww = w2 ; broadcast to 128 partitions
    aw = stat1.tile([8, 2], F32)
    nc.vector.tensor_tensor(aw[:, 0:1], lg[:, 1:2], dsc[:, 1:2], op=ALU.mult)
    nc.vector.tensor_tensor(aw[:, 0:1], aw[:, 0:1], lg[:, 0:1], op=ALU.add)
    nc.vector.tensor_copy(aw[:, 1:2], lg[:, 2:3])
    awT_ps = ps_small.tile([2, 8], F32, tag="sm", name="awT_ps")
    nc.tensor.transpose(awT_ps[:], aw[:], ident[0:8, 0:8])
    awT = stat1.tile([2, 8], F32)
    nc.scalar.copy(awT[:], awT_ps[:])
    aw_flat = stat1.tile([1, 16], F32)
    nc.scalar.dma_start(aw_flat[:], awT[:])
    abc = stat1.tile([128, 8], F32)
    nc.gpsimd.partition_broadcast(abc[:], aw_flat[:, 0:8])
    wbc = stat1.tile([128, 8], F32)
    nc.gpsimd.partition_broadcast(wbc[:], aw_flat[:, 8:16])

    # ======== Phase 4b + 5: scaled attention + output projection ========
    with tc.tile_pool(name="ph4", bufs=2) as ph4, \
         tc.tile_pool(name="o1pool", bufs=10) as o1pool:
        o1_tiles = {}
        for j in range(NTASK):
            for h in range(H):
                mm_sb = ph4.tile([128, 128], F32R, tag="mmsb", name="mm_sb")
                nc.vector.tensor_scalar(mm_sb[:], mm_raw[(h, j)][:],
                                        abc[:, h:h + 1], None, ALU.mult)
                mv_sb = ph4.tile([1, 128], F32R, tag="mvsb", name="mv_sb")
                nc.vector.tensor_scalar(mv_sb[:], mv_raw[(h, j)][:],
                                        wbc[0:1, h:h + 1], None, ALU.mult)

                # q_ratio row for this (h, j): [1, 512]
                c0 = 4 * j * 8 + h
                wq_ps = ps_small.tile([4, 128], F32, tag="sm", name="wq_ps")
                nc.tensor.transpose(wq_ps[:], qr[:, c0:c0 + 25:8], ident[:])
                wq4 = ph4.tile([4, 128], F32R, tag="wq4", name="wq4")
                nc.scalar.copy(wq4[:], wq_ps[:])
                wqr = ph4.tile([1, 512], F32R, tag="wqr", name="wqr")
                nc.scalar.dma_start(wqr[:], wq4[:])

                fqTs = ph4.tile([128, 512], F32R, tag="fqTs", name="fqTs")
                for ti in range(4):
                    t = 4 * j + ti
                    sl = slice(t * DIM + h * 128, t * DIM + h * 128 + 128)
                    qsT_ps = ps_fc.tile([128, 128], F32, tag="fc128",
                                        name="qsT_ps")
                    nc.tensor.transpose(qsT_ps[:], Fq[:, sl], ident[:])
                    nc.scalar.copy(fqTs[:, ti * 128:(ti + 1) * 128], qsT_ps[:])

                o1_ps = ps_o1.tile([128, 512], F32, tag="o1", name="o1_ps")
                nc.tensor.matmul(o1_ps[:], mm_sb[:], fqTs[:], start=True,
                                 stop=False)
                nc.tensor.matmul(o1_ps[:], mv_sb[:], wqr[:],
                                 start=False, stop=True)
                o1 = o1pool.tile([128, 512], F32R, tag="o1sb", name="o1_sb")
                nc.vector.tensor_copy(o1[:], o1_ps[:])
                o1_tiles[(h, j)] = o1

            # ---- output projection for this task ----
            for t in range(4 * j, 4 * j + 4):
                ti = t % 4
                for half in range(2):
                    o = half * 512
                    op_ps = ps_proj.tile([128, 512], F32, tag="proj",
                                         name="op_ps")
                    for h in range(H):
                        nc.tensor.matmul(
                            op_ps[:],
                            o1_tiles[(h, j)][:, ti * 128:(ti + 1) * 128],
                            WoT[:, h * DIM + o: h * DIM + o + 512],
                            start=(h == 0), stop=False)
                    nc.tensor.matmul(op_ps[:], onesr[:, 0:128],
                                     bout[:, o:o + 512],
                                     start=False, stop=True)
                    ysb = ph4.tile([128, 512], F32, tag="ysb", name="ysb")
                    nc.vector.tensor_copy(ysb[:], op_ps[:])
                    nc.sync.dma_start(y[t * 128:(t + 1) * 128, o:o + 512],
                                      ysb[:])


_BUILT = {}


def _build(n_cores=N_CORES):
    if n_cores in _BUILT:
        return _BUILT[n_cores]
    nc = bacc.Bacc("TRN2", target_bir_lowering=False, debug=False,
                   num_devices=n_cores)
    in_specs = [
        ("xn_q", [T, DIM]), ("xn_k", [T, DIM]), ("xn_v", [T, DIM]),
        ("xT_q", [128, NT * DIM]), ("xT_k", [128, NT * DIM]),
        ("xT_v", [128, NT * DIM]),
        ("Wp", [128, 8 * DIM]), ("WoT", [128, 8 * DIM]),
        ("negu", [1, DIM]), ("vrow", [1, DIM]), ("bout", [1, DIM]),
        ("ones", [128, 128]), ("ident", [128, 128]), ("mask", [128, 1024]),
        ("wp1T", [128, 256]), ("wp2T", [128, 3]), ("b1row", [1, 128]),
        ("gbc", [8, 128]), ("bbc", [8, 128]), ("b2bc", [8, 3]),
    ]
    in_aps = [nc.dram_tensor(n, s, F32, kind="ExternalInput").ap()
              for n, s in in_specs]
    y_ap = nc.dram_tensor("y", [T, DIM], F32, kind="ExternalOutput").ap()
    with tile.TileContext(nc) as tc:
        attn_kernel(tc, [y_ap], in_aps, n_cores=n_cores)
    nc.compile()
    _BUILT[n_cores] = nc
    return nc


def kernel(q, k, v, ln_g, ln_b, w_in, wp_w1, wp_b1, wp_ln_g, wp_ln_b,
           wp_w2, wp_b2, w_out, b_out):
    q = np.asarray(q, dtype=np.float32)
    k = np.asarray(k, dtype=np.float32)
    v = np.asarray(v, dtype=np.float32)
    ln_g = np.asarray(ln_g, np.float32); ln_b = np.asarray(ln_b, np.float32)
    w_in = np.asarray(w_in, np.float32); w_out = np.asarray(w_out, np.float32)
    b_out = np.asarray(b_out, np.float32)
    wp_w1 = np.asarray(wp_w1, np.float32); wp_b1 = np.asarray(wp_b1, np.float32)
    wp_ln_g = np.asarray(wp_ln_g, np.float32)
    wp_ln_b = np.asarray(wp_ln_b, np.float32)
    wp_w2 = np.asarray(wp_w2, np.float32); wp_b2 = np.asarray(wp_b2, np.float32)

    # host weight prep (folded layernorm)
    W = w_in.T                                     # [DIM, HD]
    Wp = (ln_g[:, None] * W)
    negu = -(ln_g @ W)[None, :]
    vrow = (ln_b @ W)[None, :]
    Wp_t = np.ascontiguousarray(
        Wp.reshape(8, 128, 2, 512).transpose(1, 0, 2, 3)).reshape(128, -1)
    WoT = np.ascontiguousarray(
        w_out.T.reshape(8, 128, DIM).transpose(1, 0, 2)).reshape(128, -1)
    shared = {
        "Wp": Wp_t, "WoT": WoT, "negu": negu, "vrow": vrow,
        "bout": b_out[None, :],
        "ones": np.ones((128, 128), np.float32),
        "ident": np.eye(128, dtype=np.float32),
        "mask": np.tile((1.0 - np.eye(128)).astype(np.float32), (1, 8)),
        "wp1T": np.ascontiguousarray(wp_w1.T.reshape(2, 128, 128)
                                     .transpose(1, 0, 2)).reshape(128, 256),
        "wp2T": np.ascontiguousarray(wp_w2.T),
        "b1row": wp_b1[None, :],
        "gbc": np.tile(wp_ln_g[None, :], (8, 1)),
        "bbc": np.tile(wp_ln_b[None, :], (8, 1)),
        "b2bc": np.tile(wp_b2[None, :], (8, 1)),
    }
    shared = {kk: np.ascontiguousarray(vv, np.float32)
              for kk, vv in shared.items()}

    qf = q.reshape(QB * N, DIM)
    kf = k.reshape(QB * N, DIM)
    vf = v.reshape(QB * N, DIM)
    in_maps = []
    for c in range(N_CORES):
        sl = slice(c * T, (c + 1) * T)
        m = dict(shared)
        for nm, arr in (("q", qf[sl]), ("k", kf[sl]), ("v", vf[sl])):
            m[f"xn_{nm}"] = np.ascontiguousarray(arr)
            m[f"xT_{nm}"] = np.ascontiguousarray(
                arr.reshape(NT, 128, 8, 128).transpose(3, 0, 2, 1)
            ).reshape(128, NT * DIM)
        in_maps.append(m)

    nc = _build()
    res = bass_utils.run_bass_kernel_spmd(nc, in_maps,
                                          core_ids=list(range(N_CORES)))
    global LAST_RESULTS
    LAST_RESULTS = res
    out = np.concatenate([r["y"] for r in res.results], axis=0)
    return out.reshape(QB, N, DIM)


LAST_RESULTS = None



# revision 27
# speedup vs baseline: 1.2856x; 1.2856x over previous
"""Trainium2 Bass kernel for nn_Attention_9096740733536 (sparse_attention).

Sharding: data-parallel over the QB (task) dim across 8 cores (2 tasks/core),
one mid-kernel AllReduce of [feat_corr partials | q_global | k_global] sums.
The attention math is algebraically collapsed: mixed scores are linear (no
softmax), so
  out[h,q] = alpha_h*(Fq/qn) @ ((Fk/kn)^T @ Fv) + ww_h*q_ratio (x) (kr^T Fv)
with 128x128 inner matrices instead of 512x512 score matrices, and layernorm
is folded into the input projection via a merged contraction-2 PSUM rank-1.
All heavy matmuls/transposes run in bf16 (1 cyc/row on PE for any width),
inputs/weights are staged in bf16 on the host (halves HBM traffic), and the
elementwise work is spread across ACT/DVE with wide batched instructions.
"""
import numpy as np
import ml_dtypes
from contextlib import ExitStack

import concourse.bass as bass
import concourse.tile as tile
from concourse import bacc, mybir
from concourse import bass_utils
from concourse._compat import with_exitstack

F32 = mybir.dt.float32
BF16 = mybir.dt.bfloat16
AF = mybir.ActivationFunctionType
ALU = mybir.AluOpType
AX = mybir.AxisListType

H, D, DIM = 8, 128, 1024
QB, N = 16, 512
N_CORES = 8
T = QB * N // N_CORES          # 1024 tokens per core
NT = T // 128                  # 8 token tiles per core
NTASK = T // N                 # 2 tasks per core
LN_EPS = 1e-5
TOK_ALL = float(QB * N)


@with_exitstack
def attn_kernel(ctx: ExitStack, tc: tile.TileContext, outs, ins, n_cores=N_CORES):
    nc = tc.nc
    y = outs[0]
    (xn_q, xn_k, xn_v, xT_q, xT_k, xT_v, Wp_d, WoT_d, nv2_d, bout_d,
     onesb_d, onesf_d, identb_d, identf_d, mask_d, wp1T_d, wp2T_d, b1_d,
     gbc_d, bbc_d, b2bc_d) = ins

    consts = ctx.enter_context(tc.tile_pool(name="consts", bufs=1))
    fpool = ctx.enter_context(tc.tile_pool(name="fpool", bufs=1))
    stat1 = ctx.enter_context(tc.tile_pool(name="stat1", bufs=1))
    dram = ctx.enter_context(tc.tile_pool(name="dram", bufs=1, space="DRAM"))

    ps_proj = ctx.enter_context(tc.tile_pool(name="ps_proj", bufs=2, space="PSUM"))
    ps_fc = ctx.enter_context(tc.tile_pool(name="ps_fc", bufs=1, space="PSUM"))
    ps_tr = ctx.enter_context(tc.tile_pool(name="ps_tr", bufs=1, space="PSUM"))
    ps_mv = ctx.enter_context(tc.tile_pool(name="ps_mv", bufs=1, space="PSUM"))
    ps_o1 = ctx.enter_context(tc.tile_pool(name="ps_o1", bufs=2, space="PSUM"))
    ps_sm = ctx.enter_context(tc.tile_pool(name="ps_sm", bufs=1, space="PSUM"))

    # one shared PSUM bank for all small accumulators, carved manually
    psc = ps_sm.tile([128, 512], F32, tag="sm")
    trp_aps = [psc[0:2, i * 64:(i + 1) * 64].bitcast(BF16) for i in range(3)]
    qrT_ap = psc[0:64, 192:256].bitcast(BF16)       # [64, 128] bf16
    gk_ap = psc[:, 256:272]                          # [128, 16] f32
    ss_ap = psc[0:8, 272:280]                        # [8, 8]
    h1_ap = psc[0:8, 280:408]                        # [8, 128]
    h1T_ap = psc[:, 408:416]                         # [128, 8]
    lg_ap = psc[0:8, 416:419]                        # [8, 3]
    awTa_ap = psc[0:1, 419:427]                      # [1, 8]
    awTw_ap = psc[0:1, 427:435]                      # [1, 8]

    # ---- small constants (long-lived) ----
    identb = consts.tile([128, 128], BF16)
    nc.scalar.dma_start(identb[:], identb_d[:])
    onesb = consts.tile([128, 8], BF16)
    nc.scalar.dma_start(onesb[:], onesb_d[:])
    nv2 = consts.tile([2, DIM], BF16)
    nc.scalar.dma_start(nv2[:], nv2_d[:])
    # phase-3-only constants are DMA'd later (phase 2b) to keep the ACT
    # queue clear during startup
    identf = consts.tile([8, 8], F32)
    onesf = consts.tile([128, 8], F32)
    ones8f = consts.tile([1, 8], F32)
    mask_nd = consts.tile([128, H * 128], F32)
    wp1T = consts.tile([128, 256], F32)
    wp2T = consts.tile([128, 3], F32)
    b1row = consts.tile([1, 128], F32)
    gbc = consts.tile([8, 128], F32)
    bbc = consts.tile([8, 128], F32)
    b2bc = consts.tile([8, 3], F32)
    ybias = consts.tile([128, DIM], F32)
    bout = consts.tile([1, DIM], F32)
    eps = consts.tile([128, 1], F32)
    nc.vector.memset(eps[:], LN_EPS)

    # ---- persistent F tensors: [128 tok, t*1024 + h*128 + d] bf16 ----
    late = ctx.enter_context(tc.tile_pool(name="late", bufs=1))
    WoT = late.tile([128, 8 * DIM], BF16)
    Fq = fpool.tile([128, NT * DIM], BF16)
    Fk = fpool.tile([128, NT * DIM], BF16)
    Fv = fpool.tile([128, NT * DIM], BF16)

    xns = [xn_q, xn_k, xn_v]
    xTs = [xT_q, xT_k, xT_v]
    Fs = [Fq, Fk, Fv]

    # ======== Phase 1: folded-LN projection + interleaved F stats ========
    qss = stat1.tile([128, 64], F32)   # col t*8+h : sumsq over d of Fq
    qsm = stat1.tile([128, 64], F32)   # sums over d
    kss = stat1.tile([128, 64], F32)
    ksm = stat1.tile([128, 64], F32)
    qmean = stat1.tile([128, 64], F32)
    qninv = stat1.tile([128, 64], F32)
    kninv = stat1.tile([128, 64], F32)
    qr = stat1.tile([128, 64], F32)
    kr = stat1.tile([128, 64], F32)
    qrb = stat1.tile([128, 64], BF16)
    krb = stat1.tile([128, 64], BF16)
    rscr = stat1.tile([128, 96], F32)  # ratio-chain scratch (3x32 per half)
    ar_in_fc = dram.tile([128, H * 128], F32)
    ar_out_fc = dram.tile([128, H * 128], F32)
    ar_in_gk = dram.tile([128, 16], F32)
    ar_out_gk = dram.tile([128, 16], F32)
    gk_ps = gk_ap
    qcpool = ctx.enter_context(tc.tile_pool(name="qcpool", bufs=1))
    qc_tiles = {}

    def derived(ss, sm, ninv, ratio, s):
        # ninv = 1/sqrt(ss); var = ss/127 - sm^2/(128*127)
        # ratio = 2*min(var,1)/(var+1)
        w = s.stop - s.start
        nc.scalar.activation(ninv[:, s], ss[:, s], AF.Sqrt)
        nc.vector.reciprocal(ninv[:, s], ninv[:, s])
        t1 = rscr[:, 0:w]
        nc.vector.tensor_tensor(t1, sm[:, s], sm[:, s], op=ALU.mult)
        nc.vector.tensor_scalar_mul(t1, t1, 1.0 / (D * (D - 1)))
        t2 = rscr[:, w:2 * w]
        nc.vector.tensor_scalar_mul(t2, ss[:, s], 1.0 / (D - 1))
        var = rscr[:, 2 * w:3 * w]
        nc.vector.tensor_tensor(var, t2, t1, op=ALU.subtract)
        nc.vector.tensor_scalar(t1, var, 1.0, 2.0, ALU.min, ALU.mult)
        nc.vector.tensor_scalar_add(t2, var, 1.0)
        nc.vector.reciprocal(t2, t2)
        nc.vector.tensor_tensor(ratio[:, s], t1, t2, op=ALU.mult)

    with tc.tile_pool(name="ph1", bufs=1) as ph1, \
         tc.tile_pool(name="xpool", bufs=3) as xpool, \
         tc.tile_pool(name="sqpool", bufs=2) as sqpool, \
         tc.tile_pool(name="spool", bufs=3) as spool:
        Wp = ph1.tile([128, 8 * DIM], BF16)
        xT_0 = xpool.tile([128, DIM], BF16, tag="xT")
        nc.gpsimd.dma_start(xT_0[:], xTs[0][:, 0:DIM])
        for s in range(8):
            eng = nc.gpsimd if s % 2 == 0 else nc.scalar
            eng.dma_start(Wp[:, s * DIM:(s + 1) * DIM],
                          Wp_d[:, s * DIM:(s + 1) * DIM])
        warm = {}
        for half in range(2):
            o = half * 512
            acc = ps_proj.tile([128, 512], F32, tag="proj")
            for s in range(8):
                nc.tensor.matmul(acc[:], xT_0[:, s * 128:(s + 1) * 128],
                                 Wp[:, s * DIM + o: s * DIM + o + 512],
                                 start=(s == 0), stop=False,
                                 skip_group_check=True)
            warm[half] = acc
        for t in range(NT):
            st = spool.tile([128, 12], F32, tag="st")
            bn6 = spool.tile([128, 36], F32, tag="bn6")
            rsig = spool.tile([128, 3], F32, tag="rsig")
            for i in range(3):
                xn = xpool.tile([128, DIM], BF16, tag="xn")
                nc.sync.dma_start(xn[:], xns[i][t * 128:(t + 1) * 128, :])
                nc.vector.bn_stats(bn6[:, i * 12:i * 12 + 6], xn[:, 0:512])
                nc.vector.bn_stats(bn6[:, i * 12 + 6:i * 12 + 12],
                                   xn[:, 512:1024])
                nc.vector.bn_aggr(st[:, 2 * i:2 * i + 2],
                                  bn6[:, i * 12:i * 12 + 12])
            # interleave (mu_i, sig_i) into cols 6..12
            nc.vector.tensor_copy(st[:, 6:12:2], st[:, 0:6:2])
            nc.scalar.activation(st[:, 7:12:2], st[:, 1:6:2], AF.Sqrt,
                                 bias=eps[:])
            nc.vector.reciprocal(rsig[:], st[:, 7:12:2])
            stb = spool.tile([128, 6], BF16, tag="stb")
            nc.vector.tensor_copy(stb[:], st[:, 6:12])
            rows = []
            for i in range(3):
                nc.tensor.matmul(trp_aps[i], stb[:, 2 * i:2 * i + 2],
                                 identb[:], is_transpose=True,
                                 skip_group_check=True)
                r = spool.tile([2, 128], BF16, tag=f"rows{i}", name="rows")
                nc.vector.tensor_copy(r[:], trp_aps[i])
                rows.append(r)
            for i in range(3):
                if t == 0 and i == 0:
                    xT_t = xT_0
                else:
                    xT_t = xpool.tile([128, DIM], BF16, tag="xT")
                    nc.gpsimd.dma_start(xT_t[:],
                                        xTs[i][:, t * DIM:(t + 1) * DIM])
                for half in range(2):
                    o = half * 512
                    if t == 0 and i == 0:
                        acc = warm[half]
                    else:
                        acc = ps_proj.tile([128, 512], F32, tag="proj")
                        for s in range(8):
                            nc.tensor.matmul(
                                acc[:], xT_t[:, s * 128:(s + 1) * 128],
                                Wp[:, s * DIM + o: s * DIM + o + 512],
                                start=(s == 0), stop=False)
                    nc.tensor.matmul(acc[:], rows[i][:], nv2[:, o:o + 512],
                                     start=False, stop=True,
                                     skip_group_check=(t == 0 and i == 0))
                    dst = Fs[i][:, t * DIM + o: t * DIM + o + 512]
                    nc.scalar.activation(dst, acc[:], AF.Identity,
                                         scale=rsig[:, i:i + 1])
            # ---- interleaved per-tile F stats (run on DVE/ACT slack) ----
            Fq_t = Fq[:, t * DIM:(t + 1) * DIM]
            Fk_t = Fk[:, t * DIM:(t + 1) * DIM]
            c8 = slice(t * 8, (t + 1) * 8)
            nc.vector.reduce_sum(qsm[:, c8],
                                 Fq_t.rearrange("p (h d) -> p h d", h=8),
                                 axis=AX.X)
            nc.vector.reduce_sum(ksm[:, c8],
                                 Fk_t.rearrange("p (h d) -> p h d", h=8),
                                 axis=AX.X)
            sq = sqpool.tile([128, DIM], BF16, tag="sq")
            nc.scalar.activation(sq[:], Fq_t, AF.Square)
            nc.vector.reduce_sum(qss[:, c8],
                                 sq[:].rearrange("p (h d) -> p h d", h=8),
                                 axis=AX.X)
            sk = sqpool.tile([128, DIM], BF16, tag="sk")
            nc.scalar.activation(sk[:], Fk_t, AF.Square)
            nc.vector.reduce_sum(kss[:, c8],
                                 sk[:].rearrange("p (h d) -> p h d", h=8),
                                 axis=AX.X)
            # ---- per-task ratio/norm chains + qc + gk once available ----
            if t % 4 == 3:
                jh = t // 4
                s4 = slice(jh * 32, jh * 32 + 32)
                nc.vector.tensor_scalar_mul(qmean[:, s4], qsm[:, s4], 1.0 / D)
                derived(qss, qsm, qninv, qr, s4)
                derived(kss, ksm, kninv, kr, s4)
                # store qrb with (h, ti)-permuted columns so the qrT
                # transpose yields contiguous (j, h, ti) rows
                nc.vector.tensor_copy(
                    qrb[:, s4].rearrange("p (h ti) -> p h ti", h=8, ti=4),
                    qr[:, s4].rearrange("p (ti h) -> p ti h", ti=4,
                                        h=8).transpose([0, 2, 1]))
                nc.vector.tensor_copy(krb[:, s4], kr[:, s4])
                for tt in range(4 * jh, 4 * jh + 4):
                    Fq_tt = Fq[:, tt * DIM:(tt + 1) * DIM]
                    qc = qcpool.tile([128, DIM], BF16, tag=f"qc{tt}",
                                     name="qc")
                    qmb = qmean[:, tt * 8:(tt + 1) * 8].unsqueeze(2) \
                        .broadcast_to([128, 8, 128])
                    nc.vector.tensor_tensor(
                        qc[:].rearrange("p (h d) -> p h d", h=8),
                        Fq_tt.rearrange("p (h d) -> p h d", h=8),
                        qmb, op=ALU.subtract)
                    qc_tiles[tt] = qc

    # global q/k sums from raw F. PSUM start_tensor_calc marks the WHOLE
    # bank pending-zero, so each column chain must run start-to-stop without
    # any other group starting in this bank in between (h-outer, t-inner).
    for h in range(H):
        for tt in range(NT):
            sl = slice(tt * DIM + h * 128, tt * DIM + h * 128 + 128)
            nc.tensor.matmul(gk_ps[:, h:h + 1], Fq[:, sl], onesb[:, 0:1],
                             start=(tt == 0), stop=(tt == NT - 1),
                             skip_group_check=True)
        for tt in range(NT):
            sl = slice(tt * DIM + h * 128, tt * DIM + h * 128 + 128)
            nc.tensor.matmul(gk_ps[:, 8 + h:9 + h], Fk[:, sl], onesb[:, 0:1],
                             start=(tt == 0), stop=(tt == NT - 1),
                             skip_group_check=True)

    # ======== Phase 2b: norm scaling, feat_corr, AR payload ========
    nc.scalar.dma_start(identf[:], identf_d[:])
    nc.scalar.dma_start(onesf[:], onesf_d[:])
    nc.scalar.dma_start(ones8f[:], onesf_d[0:1, :])
    nc.scalar.dma_start(mask_nd[:], mask_d[:])
    nc.scalar.dma_start(wp1T[:], wp1T_d[:])
    nc.scalar.dma_start(wp2T[:], wp2T_d[:])
    nc.scalar.dma_start(b1row[:], b1_d[:])
    nc.scalar.dma_start(gbc[:], gbc_d[:])
    nc.scalar.dma_start(bbc[:], bbc_d[:])
    nc.scalar.dma_start(b2bc[:], b2bc_d[:])
    nc.scalar.dma_start(bout[:], bout_d[:])
    nc.gpsimd.partition_broadcast(ybias[:], bout[:])
    for s in range(8):
        nc.scalar.dma_start(WoT[:, s * DIM:(s + 1) * DIM],
                            WoT_d[:, s * DIM:(s + 1) * DIM])

    # in-place norm scaling (after gk/qc/stat reads of raw F)
    for t in range(NT):
        Fq_t = Fq[:, t * DIM:(t + 1) * DIM]
        Fk_t = Fk[:, t * DIM:(t + 1) * DIM]
        knb = kninv[:, t * 8:(t + 1) * 8].unsqueeze(2) \
            .broadcast_to([128, 8, 128])
        nc.vector.tensor_tensor(Fk_t.rearrange("p (h d) -> p h d", h=8),
                                Fk_t.rearrange("p (h d) -> p h d", h=8),
                                knb, op=ALU.mult)
        qnb = qninv[:, t * 8:(t + 1) * 8].unsqueeze(2) \
            .broadcast_to([128, 8, 128])
        nc.vector.tensor_tensor(Fq_t.rearrange("p (h d) -> p h d", h=8),
                                Fq_t.rearrange("p (h d) -> p h d", h=8),
                                qnb, op=ALU.mult)

    with tc.tile_pool(name="ph2", bufs=2) as ph2:
        # gk AllReduce first: the (long) weight-predictor chain overlaps
        # the feat_corr AllReduce flight
        gk_sb = ph2.tile([128, 16], F32, tag="gksb", name="gk_sb")
        nc.scalar.copy(gk_sb[:], gk_ps[:])
        nc.sync.dma_start(ar_in_gk[:], gk_sb[:])
        if n_cores > 1:
            nc.gpsimd.collective_compute(
                "AllReduce", ALU.add,
                replica_groups=[list(range(n_cores))],
                ins=[ar_in_gk.opt()], outs=[ar_out_gk.opt()])
        else:
            nc.sync.dma_start(ar_out_gk[:], ar_in_gk[:])
        arg = late.tile([128, 16], F32)
        nc.sync.dma_start(arg[:], ar_out_gk[:])

        # feat_corr partials, 4 heads per PSUM bank tile -> ar_in_fc
        for g in range(2):
            fc_ps = ps_fc.tile([128, 512], F32, tag="fc", name="fc_ps")
            for hh in range(4):
                h = g * 4 + hh
                for t in range(NT):
                    qsl = qc_tiles[t][:, h * 128:(h + 1) * 128]
                    nc.tensor.matmul(fc_ps[:, hh * 128:(hh + 1) * 128],
                                     qsl, qsl, start=(t == 0),
                                     stop=(t == NT - 1),
                                     skip_group_check=True)
            fc_sb = ph2.tile([128, 512], F32, tag="fcsb", name="fc_sb")
            nc.vector.tensor_copy(fc_sb[:], fc_ps[:])
            nc.sync.dma_start(ar_in_fc[:, g * 512:(g + 1) * 512], fc_sb[:])
    if n_cores > 1:
        nc.gpsimd.collective_compute(
            "AllReduce", ALU.add,
            replica_groups=[list(range(n_cores))],
            ins=[ar_in_fc.opt()], outs=[ar_out_fc.opt()])
    else:
        nc.sync.dma_start(ar_out_fc[:], ar_in_fc[:])
    ar = late.tile([128, H * 128], F32)
    nc.sync.dma_start(ar[:], ar_out_fc[:])

    # -- phase 3 part A (gk-dependent): weight-predictor input + h1 --
    featsq = stat1.tile([128, 8], F32)
    nc.vector.tensor_scalar_mul(featsq[:], arg[:, 0:8], 1.0 / TOK_ALL)
    featsk = stat1.tile([128, 8], F32)
    nc.vector.tensor_scalar_mul(featsk[:], arg[:, 8:16], 1.0 / TOK_ALL)
    nc.tensor.matmul(h1_ap, featsq[:], wp1T[:, 0:128], start=True,
                     stop=False, skip_group_check=True)
    nc.tensor.matmul(h1_ap, featsk[:], wp1T[:, 128:256], start=False,
                     stop=False, skip_group_check=True)
    nc.tensor.matmul(h1_ap, ones8f[:], b1row[:], start=False, stop=True,
                     skip_group_check=True)
    h1 = stat1.tile([8, 128], F32)
    nc.scalar.copy(h1[:], h1_ap)

    # ======== Phase 4a: AR-independent PE work (covers AR flight) ========
    # mm/mv packed 4 heads per PSUM bank; raw (unscaled) eviction to SBUF.
    attn = ctx.enter_context(tc.tile_pool(name="attn", bufs=1))
    mm_raw = {}
    mv_raw = {}
    for j in range(NTASK):
        for g in range(2):
            mm_ps = ps_fc.tile([128, 512], F32, tag="fc", name="mm_ps")
            mv_ps = ps_mv.tile([1, 512], F32, tag="mv", name="mv_ps")
            for hh in range(4):
                h = g * 4 + hh
                for ti in range(4):
                    t = 4 * j + ti
                    sl = slice(t * DIM + h * 128, t * DIM + h * 128 + 128)
                    nc.tensor.matmul(mm_ps[:, hh * 128:(hh + 1) * 128],
                                     Fk[:, sl], Fv[:, sl],
                                     start=(ti == 0), stop=(ti == 3),
                                     skip_group_check=True)
                    nc.tensor.matmul(mv_ps[:, hh * 128:(hh + 1) * 128],
                                     krb[:, t * 8 + h:t * 8 + h + 1],
                                     Fv[:, sl], start=(ti == 0),
                                     stop=(ti == 3), skip_group_check=True)
            mm = attn.tile([128, 512], BF16, tag=f"mm{j}{g}", name="mm")
            nc.scalar.copy(mm[:], mm_ps[:])
            mv = attn.tile([1, 512], BF16, tag=f"mv{j}{g}", name="mv")
            nc.scalar.copy(mv[:], mv_ps[:])
            mm_raw[(j, g)] = mm
            mv_raw[(j, g)] = mv

    # -- phase 3 part B: h1 layernorm + relu (DVE/ACT, overlaps fqT) --
    sq3 = late.tile([128, H * 128], BF16)   # write-only square scratch
    w_mu = stat1.tile([8, 4], F32)
    nc.vector.reduce_sum(w_mu[:, 0:1], h1[:], axis=AX.X)
    nc.vector.tensor_scalar_mul(w_mu[:, 0:1], w_mu[:, 0:1], 1.0 / D)
    nc.scalar.activation(sq3[0:8, 0:128], h1[:], AF.Square,
                         accum_out=w_mu[:, 1:2])
    nc.vector.tensor_scalar_mul(w_mu[:, 1:2], w_mu[:, 1:2], 1.0 / D)
    nc.vector.tensor_tensor(w_mu[:, 2:3], w_mu[:, 0:1], w_mu[:, 0:1],
                            op=ALU.mult)
    nc.vector.tensor_tensor(w_mu[:, 2:3], w_mu[:, 1:2], w_mu[:, 2:3],
                            op=ALU.subtract)
    nc.scalar.activation(w_mu[:, 3:4], w_mu[:, 2:3], AF.Sqrt, bias=eps[0:8, :])
    nc.vector.reciprocal(w_mu[:, 3:4], w_mu[:, 3:4])
    h1n = stat1.tile([8, 128], F32)
    nc.vector.tensor_scalar(h1n[:], h1[:], w_mu[:, 0:1], w_mu[:, 3:4],
                            ALU.subtract, ALU.mult)
    nc.vector.tensor_tensor(h1n[:], h1n[:], gbc[:], op=ALU.mult)
    nc.vector.tensor_tensor(h1n[:], h1n[:], bbc[:], op=ALU.add)
    nc.vector.tensor_scalar_max(h1n[:], h1n[:], 0.0)

    # Fq^T tiles (scaled Fq, bf16): 8 transposes (2 head-groups) per PSUM
    # bank tile + 1 wide copy
    fqTs = {}
    for j in range(NTASK):
        for g in range(4):
            tr_ps = ps_tr.tile([128, 1024], BF16, tag="tr", name="tr_ps")
            for hh in range(2):
                h = g * 2 + hh
                for ti in range(4):
                    t = 4 * j + ti
                    sl = slice(t * DIM + h * 128, t * DIM + h * 128 + 128)
                    nc.tensor.transpose(
                        tr_ps[:, hh * 512 + ti * 128:hh * 512 + ti * 128 + 128],
                        Fq[:, sl], identb[:])
            fqT = attn.tile([128, 1024], BF16, tag=f"fqT{j}{g}", name="fqT")
            nc.vector.tensor_copy(fqT[:], tr_ps[:])
            fqTs[(j, g)] = fqT

    # -- phase 3 part C: second predictor layer + softmax --
    nc.tensor.matmul(h1T_ap, h1n[:], identf[:], is_transpose=True,
                     skip_group_check=True)
    h1T = stat1.tile([128, 8], F32)
    nc.scalar.copy(h1T[:], h1T_ap)
    nc.tensor.matmul(lg_ap, h1T[:], wp2T[:], start=True, stop=True,
                     skip_group_check=True)
    lg = stat1.tile([8, 8], F32)
    nc.scalar.copy(lg[:, 0:3], lg_ap)
    nc.vector.tensor_tensor(lg[:, 0:3], lg[:, 0:3], b2bc[:], op=ALU.add)
    # logits are O(1): skip the (mathematically redundant) max-subtraction
    nc.scalar.activation(lg[:, 0:3], lg[:, 0:3], AF.Exp)
    nc.vector.reduce_sum(lg[:, 4:5], lg[:, 0:3], axis=AX.X)
    nc.vector.reciprocal(lg[:, 4:5], lg[:, 4:5])
    nc.vector.tensor_scalar(lg[:, 0:3], lg[:, 0:3], lg[:, 4:5], None, ALU.mult)

    # q_ratio rows: one whole-tile transpose with (j,h,ti)-reordered rows,
    # then contiguous-partition flatten DMAs (partition-strided DMA slices
    # are not supported)
    nc.tensor.matmul(qrT_ap, qrb[:], identb[:], is_transpose=True,
                     skip_group_check=True)
    qrT_sb = stat1.tile([64, 128], BF16)
    nc.vector.tensor_copy(qrT_sb[:], qrT_ap)
    # q_ratio rows flattened pre-AR (DMAs fly during the collective)
    wqrs = {}
    for j in range(NTASK):
        for h in range(H):
            r0 = j * 32 + h * 4
            wqr = attn.tile([1, 512], BF16, tag=f"wqr{h}{j}", name="wqr")
            nc.sync.dma_start(wqr[:], qrT_sb[r0:r0 + 4, :])
            wqrs[(h, j)] = wqr

    # -- phase 3 part D (fc-dependent): decorrelation scale --
    ssq = stat1.tile([128, 8], F32)
    msk = late.tile([128, H * 128], F32)
    nc.vector.tensor_tensor(msk[:], ar[:], mask_nd[:], op=ALU.mult)
    nc.scalar.activation(sq3[:], msk[:], AF.Square, scale=1.0 / TOK_ALL)
    nc.vector.reduce_sum(ssq[:],
                         sq3[:].rearrange("p (h d) -> p h d", h=8),
                         axis=AX.X)
    nc.tensor.matmul(ss_ap, ssq[:], onesf[:], start=True, stop=True,
                     skip_group_check=True)
    dsc = stat1.tile([8, 8], F32)
    nc.scalar.activation(dsc[:, 0:1], ss_ap[0:8, 0:1], AF.Sqrt)
    nc.scalar.activation(dsc[:, 1:2], dsc[:, 0:1], AF.Exp, scale=-5.0 / (D * D))
    # alpha = w0 + w1*dsc ; ww = w2 ; broadcast to 128 partitions
    aw = stat1.tile([8, 2], F32)
    nc.vector.tensor_tensor(aw[:, 0:1], lg[:, 1:2], dsc[:, 1:2], op=ALU.mult)
    nc.vector.tensor_tensor(aw[:, 0:1], aw[:, 0:1], lg[:, 0:1], op=ALU.add)
    nc.vector.tensor_copy(aw[:, 1:2], lg[:, 2:3])
    nc.tensor.matmul(awTa_ap, aw[:, 0:1], identf[:],
                     is_transpose=True, skip_group_check=True)
    nc.tensor.matmul(awTw_ap, aw[:, 1:2], identf[:],
                     is_transpose=True, skip_group_check=True)
    awTa = stat1.tile([1, 8], F32)
    nc.scalar.copy(awTa[:], awTa_ap)
    awTw = stat1.tile([1, 8], F32)
    nc.scalar.copy(awTw[:], awTw_ap)
    abc = stat1.tile([128, 8], F32)
    nc.gpsimd.partition_broadcast(abc[:], awTa[:])
    wbc = stat1.tile([128, 8], F32)
    nc.gpsimd.partition_broadcast(wbc[:], awTw[:])

    # ======== Phase 4b + 5: scaled attention + output projection ========
    with tc.tile_pool(name="ph4", bufs=2) as ph4, \
         tc.tile_pool(name="o1pool", bufs=1) as o1pool:
        o1_tiles = {}
        mm_sb = {}
        mv_sb = {}
        for j in range(NTASK):
            # scale mm/mv by the dynamic per-head weights (batched, 4 heads)
            for g in range(2):
                msb = ph4.tile([128, 512], BF16, tag=f"mmsb{j}{g}",
                               name="mm_sb")
                ab = abc[:, g * 4:(g + 1) * 4].unsqueeze(2) \
                    .broadcast_to([128, 4, 128])
                nc.vector.tensor_tensor(
                    msb[:].rearrange("p (h d) -> p h d", h=4),
                    mm_raw[(j, g)][:].rearrange("p (h d) -> p h d", h=4),
                    ab, op=ALU.mult)
                mm_sb[(j, g)] = msb
                vsb = ph4.tile([1, 512], BF16, tag=f"mvsb{j}{g}",
                               name="mv_sb")
                wb = wbc[0:1, g * 4:(g + 1) * 4].unsqueeze(2) \
                    .broadcast_to([1, 4, 128])
                nc.vector.tensor_tensor(
                    vsb[:].rearrange("p (h d) -> p h d", h=4),
                    mv_raw[(j, g)][:].rearrange("p (h d) -> p h d", h=4),
                    wb, op=ALU.mult)
                mv_sb[(j, g)] = vsb
        for j in range(NTASK):
            for h in range(H):
                g, hh = h // 4, h % 4
                wqr = wqrs[(h, j)]
                opool = ps_o1 if h % 2 == 0 else ps_proj
                o1_ps = opool.tile([128, 512], F32,
                                   tag="o1" if h % 2 == 0 else "proj",
                                   name="o1_ps")
                nc.tensor.matmul(
                    o1_ps[:], mm_sb[(j, g)][:, hh * 128:(hh + 1) * 128],
                    fqTs[(j, h // 2)][:, (h % 2) * 512:(h % 2) * 512 + 512],
                    start=True, stop=False)
                nc.tensor.matmul(o1_ps[:],
                                 mv_sb[(j, g)][:, hh * 128:(hh + 1) * 128],
                                 wqr[:], start=False, stop=True)
                o1 = o1pool.tile([128, 512], BF16, tag=f"o1sb{h}{j}",
                                 name="o1_sb")
                if h % 2 == 0:
                    nc.scalar.copy(o1[:], o1_ps[:])
                else:
                    nc.vector.tensor_copy(o1[:], o1_ps[:])
                o1_tiles[(h, j)] = o1

        # ---- output projection, both tasks ----
        for j in range(NTASK):
            for t in range(4 * j, 4 * j + 4):
                ti = t % 4
                for half in range(2):
                    o = half * 512
                    opool2 = ps_proj if half == 0 else ps_o1
                    op_ps = opool2.tile([128, 512], F32,
                                        tag="proj" if half == 0 else "o1",
                                        name="op_ps")
                    for h in range(H):
                        nc.tensor.matmul(
                            op_ps[:],
                            o1_tiles[(h, j)][:, ti * 128:(ti + 1) * 128],
                            WoT[:, h * DIM + o: h * DIM + o + 512],
                            start=(h == 0), stop=(h == H - 1))
                    ysb = ph4.tile([128, 512], BF16, tag="ysb", name="ysb")
                    nc.vector.tensor_tensor(ysb[:], op_ps[:],
                                            ybias[:, o:o + 512], op=ALU.add)
                    nc.sync.dma_start(y[t * 128:(t + 1) * 128, o:o + 512],
                                      ysb[:])


_BUILT = {}


def _build(n_cores=N_CORES):
    if n_cores in _BUILT:
        return _BUILT[n_cores]
    nc = bacc.Bacc("TRN2", target_bir_lowering=False, debug=False,
                   num_devices=n_cores)
    in_specs = [
        ("xn_q", [T, DIM], BF16), ("xn_k", [T, DIM], BF16),
        ("xn_v", [T, DIM], BF16),
        ("xT_q", [128, NT * DIM], BF16), ("xT_k", [128, NT * DIM], BF16),
        ("xT_v", [128, NT * DIM], BF16),
        ("Wp", [128, 8 * DIM], BF16), ("WoT", [128, 8 * DIM], BF16),
        ("nv2", [2, DIM], BF16), ("bout", [1, DIM], F32),
        ("onesb", [128, 8], BF16), ("onesf", [128, 8], F32),
        ("identb", [128, 128], BF16), ("identf", [8, 8], F32),
        ("mask", [128, 1024], F32),
        ("wp1T", [128, 256], F32), ("wp2T", [128, 3], F32),
        ("b1row", [1, 128], F32),
        ("gbc", [8, 128], F32), ("bbc", [8, 128], F32), ("b2bc", [8, 3], F32),
    ]
    in_aps = [nc.dram_tensor(n, s, d, kind="ExternalInput").ap()
              for n, s, d in in_specs]
    y_ap = nc.dram_tensor("y", [T, DIM], BF16, kind="ExternalOutput").ap()
    with tile.TileContext(nc) as tc:
        attn_kernel(tc, [y_ap], in_aps, n_cores=n_cores)
    nc.compile()
    _BUILT[n_cores] = nc
    return nc


def kernel(q, k, v, ln_g, ln_b, w_in, wp_w1, wp_b1, wp_ln_g, wp_ln_b,
           wp_w2, wp_b2, w_out, b_out):
    q = np.asarray(q, dtype=np.float32)
    k = np.asarray(k, dtype=np.float32)
    v = np.asarray(v, dtype=np.float32)
    ln_g = np.asarray(ln_g, np.float32); ln_b = np.asarray(ln_b, np.float32)
    w_in = np.asarray(w_in, np.float32); w_out = np.asarray(w_out, np.float32)
    b_out = np.asarray(b_out, np.float32)
    wp_w1 = np.asarray(wp_w1, np.float32); wp_b1 = np.asarray(wp_b1, np.float32)
    wp_ln_g = np.asarray(wp_ln_g, np.float32)
    wp_ln_b = np.asarray(wp_ln_b, np.float32)
    wp_w2 = np.asarray(wp_w2, np.float32); wp_b2 = np.asarray(wp_b2, np.float32)

    bf = ml_dtypes.bfloat16

    # host weight prep (folded layernorm)
    W = w_in.T                                     # [DIM, HD]
    Wp = (ln_g[:, None] * W)
    negu = -(ln_g @ W)
    vrow = (ln_b @ W)
    nv2 = np.stack([negu, vrow], axis=0)           # [2, DIM]
    Wp_t = np.ascontiguousarray(
        Wp.reshape(8, 128, DIM).transpose(1, 0, 2)).reshape(128, -1)
    WoT = np.ascontiguousarray(
        w_out.T.reshape(8, 128, DIM).transpose(1, 0, 2)).reshape(128, -1)
    shared = {
        "Wp": Wp_t.astype(bf), "WoT": WoT.astype(bf),
        "nv2": nv2.astype(bf),
        "bout": np.ascontiguousarray(b_out[None, :], np.float32),
        "onesb": np.ones((128, 8), bf),
        "onesf": np.ones((128, 8), np.float32),
        "identb": np.eye(128).astype(bf),
        "identf": np.eye(8, dtype=np.float32),
        "mask": np.ascontiguousarray(
            np.tile((1.0 - np.eye(128)).astype(np.float32), (1, 8))),
        "wp1T": np.ascontiguousarray(wp_w1.T.reshape(2, 128, 128)
                                     .transpose(1, 0, 2)).reshape(128, 256),
        "wp2T": np.ascontiguousarray(wp_w2.T),
        "b1row": np.ascontiguousarray(wp_b1[None, :]),
        "gbc": np.ascontiguousarray(np.tile(wp_ln_g[None, :], (8, 1))),
        "bbc": np.ascontiguousarray(np.tile(wp_ln_b[None, :], (8, 1))),
        "b2bc": np.ascontiguousarray(np.tile(wp_b2[None, :], (8, 1))),
    }

    qf = q.reshape(QB * N, DIM)
    kf = k.reshape(QB * N, DIM)
    vf = v.reshape(QB * N, DIM)
    in_maps = []
    for c in range(N_CORES):
        sl = slice(c * T, (c + 1) * T)
        m = dict(shared)
        for nm, arr in (("q", qf[sl]), ("k", kf[sl]), ("v", vf[sl])):
            ab = arr.astype(bf)
            m[f"xn_{nm}"] = np.ascontiguousarray(ab)
            m[f"xT_{nm}"] = np.ascontiguousarray(
                ab.reshape(NT, 128, 8, 128).transpose(3, 0, 2, 1)
            ).reshape(128, NT * DIM)
        in_maps.append(m)

    nc = _build()
    res = bass_utils.run_bass_kernel_spmd(nc, in_maps,
                                          core_ids=list(range(N_CORES)))
    global LAST_RESULTS
    LAST_RESULTS = res
    out = np.concatenate([np.asarray(r["y"], np.float32)
                          for r in res.results], axis=0)
    return out.reshape(QB, N, DIM)


LAST_RESULTS = None


# revision 34
# speedup vs baseline: 1.3805x; 1.0739x over previous
"""Trainium2 Bass kernel for nn_Attention_9096740733536 (sparse_attention).

Sharding: data-parallel over the QB (task) dim across 8 cores (2 tasks/core),
one mid-kernel AllReduce of [feat_corr partials | q_global | k_global] sums.
The attention math is algebraically collapsed: mixed scores are linear (no
softmax), so
  out[h,q] = alpha_h*(Fq/qn) @ ((Fk/kn)^T @ Fv) + ww_h*q_ratio (x) (kr^T Fv)
with 128x128 inner matrices instead of 512x512 score matrices, and layernorm
is folded into the input projection via a merged contraction-2 PSUM rank-1.
All heavy matmuls/transposes run in bf16 (1 cyc/row on PE for any width),
inputs/weights are staged in bf16 on the host (halves HBM traffic), and the
elementwise work is spread across ACT/DVE with wide batched instructions.
"""
import numpy as np
import ml_dtypes
from contextlib import ExitStack

import concourse.bass as bass
import concourse.tile as tile
from concourse import bacc, mybir
from concourse import bass_utils
from concourse._compat import with_exitstack

F32 = mybir.dt.float32
BF16 = mybir.dt.bfloat16
AF = mybir.ActivationFunctionType
ALU = mybir.AluOpType
AX = mybir.AxisListType

H, D, DIM = 8, 128, 1024
QB, N = 16, 512
N_CORES = 8
T = QB * N // N_CORES          # 1024 tokens per core
NT = T // 128                  # 8 token tiles per core
NTASK = T // N                 # 2 tasks per core
LN_EPS = 1e-5
TOK_ALL = float(QB * N)


@with_exitstack
def attn_kernel(ctx: ExitStack, tc: tile.TileContext, outs, ins, n_cores=N_CORES):
    nc = tc.nc
    y = outs[0]
    (xn_q, xn_k, xn_v, xT_q, xT_k, xT_v, Wp_d, WoT_d, nv2_d, bout_d,
     onesb_d, onesf_d, identb_d, identf_d, mask_d, wp1T_d, wp2T_d, b1_d,
     gbc_d, bbc_d, b2bc_d) = ins

    consts = ctx.enter_context(tc.tile_pool(name="consts", bufs=1))
    fpool = ctx.enter_context(tc.tile_pool(name="fpool", bufs=1))
    stat1 = ctx.enter_context(tc.tile_pool(name="stat1", bufs=1))
    dram = ctx.enter_context(tc.tile_pool(name="dram", bufs=1, space="DRAM"))

    ps_proj = ctx.enter_context(tc.tile_pool(name="ps_proj", bufs=2, space="PSUM"))
    ps_fc = ctx.enter_context(tc.tile_pool(name="ps_fc", bufs=1, space="PSUM"))
    ps_tr = ctx.enter_context(tc.tile_pool(name="ps_tr", bufs=1, space="PSUM"))
    ps_mv = ctx.enter_context(tc.tile_pool(name="ps_mv", bufs=1, space="PSUM"))
    ps_o1 = ctx.enter_context(tc.tile_pool(name="ps_o1", bufs=2, space="PSUM"))
    ps_sm = ctx.enter_context(tc.tile_pool(name="ps_sm", bufs=1, space="PSUM"))

    # one shared PSUM bank for all small accumulators, carved manually
    psc = ps_sm.tile([128, 512], F32, tag="sm")
    trp_aps = [psc[0:2, i * 64:(i + 1) * 64].bitcast(BF16) for i in range(3)]
    qrT_ap = psc[0:64, 192:256].bitcast(BF16)       # [64, 128] bf16
    gk_ap = psc[:, 256:288]                          # [128, 32] f32 (per-task)
    ss_ap = psc[0:8, 288:296]                        # [8, 8]
    h1_ap = psc[0:8, 296:424]                        # [8, 128]
    h1T_ap = psc[:, 424:432]                         # [128, 8]
    lg_ap = psc[0:8, 432:435]                        # [8, 3]
    awTa_ap = psc[0:1, 435:443]                      # [1, 8]
    awTw_ap = psc[0:1, 443:451]                      # [1, 8]

    # ---- small constants (long-lived) ----
    identb = consts.tile([128, 128], BF16)
    nc.scalar.dma_start(identb[:], identb_d[:])
    onesb = consts.tile([128, 8], BF16)
    nc.scalar.dma_start(onesb[:], onesb_d[:])
    nv2 = consts.tile([2, DIM], BF16)
    nc.scalar.dma_start(nv2[:], nv2_d[:])
    # phase-3-only constants are DMA'd later (phase 2b) to keep the ACT
    # queue clear during startup
    identf = consts.tile([8, 8], F32)
    onesf = consts.tile([128, 8], F32)
    ones8f = consts.tile([1, 8], F32)
    mask_nd = consts.tile([128, H * 128], F32)
    wp1T = consts.tile([128, 256], F32)
    wp2T = consts.tile([128, 3], F32)
    b1row = consts.tile([1, 128], F32)
    gbc = consts.tile([8, 128], F32)
    bbc = consts.tile([8, 128], F32)
    b2bc = consts.tile([8, 3], F32)
    ybias = consts.tile([128, DIM], F32)
    bout = consts.tile([1, DIM], F32)
    eps = consts.tile([128, 1], F32)
    nc.vector.memset(eps[:], LN_EPS)

    # ---- persistent F tensors: [128 tok, t*1024 + h*128 + d] bf16 ----
    late = ctx.enter_context(tc.tile_pool(name="late", bufs=1))
    WoT = late.tile([128, 8 * DIM], BF16)
    Fq = fpool.tile([128, NT * DIM], BF16)
    Fk = fpool.tile([128, NT * DIM], BF16)
    Fv = fpool.tile([128, NT * DIM], BF16)

    xns = [xn_q, xn_k, xn_v]
    xTs = [xT_q, xT_k, xT_v]
    Fs = [Fq, Fk, Fv]

    # ======== Phase 1: folded-LN projection + interleaved F stats ========
    qss = stat1.tile([128, 64], F32)   # col t*8+h : sumsq over d of Fq
    qsm = stat1.tile([128, 64], F32)   # sums over d
    kss = stat1.tile([128, 64], F32)
    ksm = stat1.tile([128, 64], F32)
    qmean = stat1.tile([128, 64], F32)
    qninv = stat1.tile([128, 64], F32)
    kninv = stat1.tile([128, 64], F32)
    qr = stat1.tile([128, 64], F32)
    kr = stat1.tile([128, 64], F32)
    qrb = stat1.tile([128, 64], BF16)
    krb = stat1.tile([128, 64], BF16)
    rscr = stat1.tile([128, 96], F32)  # ratio-chain scratch (3x32 per half)
    ar_in_fc = dram.tile([128, H * 128], F32)
    ar_out_fc = dram.tile([128, H * 128], F32)
    ar_in_gk = dram.tile([128, 16], F32)
    ar_out_gk = dram.tile([128, 16], F32)
    gk_ps = gk_ap
    qcpool = ctx.enter_context(tc.tile_pool(name="qcpool", bufs=1))
    qc_tiles = {}

    def derived(ss, sm, ninv, ratio, s):
        # ninv = 1/sqrt(ss); var = ss/127 - sm^2/(128*127)
        # ratio = 2*min(var,1)/(var+1)
        w = s.stop - s.start
        nc.scalar.activation(ninv[:, s], ss[:, s], AF.Sqrt)
        nc.vector.reciprocal(ninv[:, s], ninv[:, s])
        t1 = rscr[:, 0:w]
        nc.vector.tensor_tensor(t1, sm[:, s], sm[:, s], op=ALU.mult)
        nc.vector.tensor_scalar_mul(t1, t1, 1.0 / (D * (D - 1)))
        t2 = rscr[:, w:2 * w]
        nc.vector.tensor_scalar_mul(t2, ss[:, s], 1.0 / (D - 1))
        var = rscr[:, 2 * w:3 * w]
        nc.vector.tensor_tensor(var, t2, t1, op=ALU.subtract)
        nc.vector.tensor_scalar(t1, var, 1.0, 2.0, ALU.min, ALU.mult)
        nc.vector.tensor_scalar_add(t2, var, 1.0)
        nc.vector.reciprocal(t2, t2)
        nc.vector.tensor_tensor(ratio[:, s], t1, t2, op=ALU.mult)

    with tc.tile_pool(name="ph1", bufs=1) as ph1, \
         tc.tile_pool(name="xpool", bufs=3) as xpool, \
         tc.tile_pool(name="sqpool", bufs=2) as sqpool, \
         tc.tile_pool(name="spool", bufs=3) as spool:
        Wp = ph1.tile([128, 8 * DIM], BF16)
        xT_0 = xpool.tile([128, DIM], BF16, tag="xT")
        nc.gpsimd.dma_start(xT_0[:], xTs[0][:, 0:DIM])
        for s in range(8):
            eng = nc.gpsimd if s % 2 == 0 else nc.scalar
            eng.dma_start(Wp[:, s * DIM:(s + 1) * DIM],
                          Wp_d[:, s * DIM:(s + 1) * DIM])
        warm = {}
        for half in range(2):
            o = half * 512
            acc = ps_proj.tile([128, 512], F32, tag="proj")
            for s in range(8):
                nc.tensor.matmul(acc[:], xT_0[:, s * 128:(s + 1) * 128],
                                 Wp[:, s * DIM + o: s * DIM + o + 512],
                                 start=(s == 0), stop=False,
                                 skip_group_check=True)
            warm[half] = acc
        for t in range(NT):
            st = spool.tile([128, 12], F32, tag="st")
            bn6 = spool.tile([128, 36], F32, tag="bn6")
            rsig = spool.tile([128, 3], F32, tag="rsig")
            for i in range(3):
                xn = xpool.tile([128, DIM], BF16, tag="xn")
                nc.sync.dma_start(xn[:], xns[i][t * 128:(t + 1) * 128, :])
                nc.vector.bn_stats(bn6[:, i * 12:i * 12 + 6], xn[:, 0:512])
                nc.vector.bn_stats(bn6[:, i * 12 + 6:i * 12 + 12],
                                   xn[:, 512:1024])
                nc.vector.bn_aggr(st[:, 2 * i:2 * i + 2],
                                  bn6[:, i * 12:i * 12 + 12])
            # interleave (mu_i, sig_i) into cols 6..12
            nc.vector.tensor_copy(st[:, 6:12:2], st[:, 0:6:2])
            nc.scalar.activation(st[:, 7:12:2], st[:, 1:6:2], AF.Sqrt,
                                 bias=eps[:])
            nc.vector.reciprocal(rsig[:], st[:, 7:12:2])
            stb = spool.tile([128, 6], BF16, tag="stb")
            nc.vector.tensor_copy(stb[:], st[:, 6:12])
            for i in range(3):
                nc.tensor.matmul(trp_aps[i], stb[:, 2 * i:2 * i + 2],
                                 identb[:], is_transpose=True,
                                 skip_group_check=True)
            rows_all = spool.tile([2, 384], BF16, tag="rows", name="rows")
            nc.vector.tensor_copy(rows_all[:],
                                  psc[0:2, 0:192].bitcast(BF16))
            rows = [rows_all[:, i * 128:(i + 1) * 128] for i in range(3)]
            for i in range(3):
                if t == 0 and i == 0:
                    xT_t = xT_0
                else:
                    xT_t = xpool.tile([128, DIM], BF16, tag="xT")
                    nc.gpsimd.dma_start(xT_t[:],
                                        xTs[i][:, t * DIM:(t + 1) * DIM])
                for half in range(2):
                    o = half * 512
                    if t == 0 and i == 0:
                        acc = warm[half]
                    else:
                        acc = ps_proj.tile([128, 512], F32, tag="proj")
                        for s in range(8):
                            nc.tensor.matmul(
                                acc[:], xT_t[:, s * 128:(s + 1) * 128],
                                Wp[:, s * DIM + o: s * DIM + o + 512],
                                start=(s == 0), stop=False)
                    nc.tensor.matmul(acc[:], rows[i], nv2[:, o:o + 512],
                                     start=False, stop=True,
                                     skip_group_check=(t == 0 and i == 0))
                    dst = Fs[i][:, t * DIM + o: t * DIM + o + 512]
                    nc.scalar.activation(dst, acc[:], AF.Identity,
                                         scale=rsig[:, i:i + 1])
            # ---- interleaved per-tile F stats (run on DVE/ACT slack) ----
            Fq_t = Fq[:, t * DIM:(t + 1) * DIM]
            Fk_t = Fk[:, t * DIM:(t + 1) * DIM]
            c8 = slice(t * 8, (t + 1) * 8)
            nc.vector.reduce_sum(qsm[:, c8],
                                 Fq_t.rearrange("p (h d) -> p h d", h=8),
                                 axis=AX.X)
            nc.vector.reduce_sum(ksm[:, c8],
                                 Fk_t.rearrange("p (h d) -> p h d", h=8),
                                 axis=AX.X)
            sq = sqpool.tile([128, DIM], BF16, tag="sq")
            nc.scalar.activation(sq[:], Fq_t, AF.Square)
            nc.vector.reduce_sum(qss[:, c8],
                                 sq[:].rearrange("p (h d) -> p h d", h=8),
                                 axis=AX.X)
            sk = sqpool.tile([128, DIM], BF16, tag="sk")
            nc.scalar.activation(sk[:], Fk_t, AF.Square)
            nc.vector.reduce_sum(kss[:, c8],
                                 sk[:].rearrange("p (h d) -> p h d", h=8),
                                 axis=AX.X)
            # ---- per-task ratio/norm chains + qc + gk once available ----
            if t % 4 == 3:
                jh = t // 4
                s4 = slice(jh * 32, jh * 32 + 32)
                nc.vector.tensor_scalar_mul(qmean[:, s4], qsm[:, s4], 1.0 / D)
                derived(qss, qsm, qninv, qr, s4)
                derived(kss, ksm, kninv, kr, s4)
                # store qrb with (h, ti)-permuted columns so the qrT
                # transpose yields contiguous (j, h, ti) rows
                nc.vector.tensor_copy(
                    qrb[:, s4].rearrange("p (h ti) -> p h ti", h=8, ti=4),
                    qr[:, s4].rearrange("p (ti h) -> p ti h", ti=4,
                                        h=8).transpose([0, 2, 1]))
                nc.vector.tensor_copy(krb[:, s4], kr[:, s4])
                for tt in range(4 * jh, 4 * jh + 4):
                    Fq_tt = Fq[:, tt * DIM:(tt + 1) * DIM]
                    qc = qcpool.tile([128, DIM], BF16, tag=f"qc{tt}",
                                     name="qc")
                    qmb = qmean[:, tt * 8:(tt + 1) * 8].unsqueeze(2) \
                        .broadcast_to([128, 8, 128])
                    nc.vector.tensor_tensor(
                        qc[:].rearrange("p (h d) -> p h d", h=8),
                        Fq_tt.rearrange("p (h d) -> p h d", h=8),
                        qmb, op=ALU.subtract)
                    qc_tiles[tt] = qc

    # global q/k sums from raw F: complete per-column chains (PSUM start
    # marks the whole bank pending-zero, chains must not interleave)
    for jh in range(NTASK):
        for h in range(H):
            for tt in range(4 * jh, 4 * jh + 4):
                sl = slice(tt * DIM + h * 128, tt * DIM + h * 128 + 128)
                nc.tensor.matmul(gk_ps[:, jh * 16 + h:jh * 16 + h + 1],
                                 Fq[:, sl], onesb[:, 0:1],
                                 start=(tt % 4 == 0), stop=(tt % 4 == 3),
                                 skip_group_check=True)
            for tt in range(4 * jh, 4 * jh + 4):
                sl = slice(tt * DIM + h * 128, tt * DIM + h * 128 + 128)
                nc.tensor.matmul(gk_ps[:, jh * 16 + 8 + h:jh * 16 + 9 + h],
                                 Fk[:, sl], onesb[:, 0:1],
                                 start=(tt % 4 == 0), stop=(tt % 4 == 3),
                                 skip_group_check=True)

    # gk reduction hoisted before the scales on the DVE queue so the gk
    # AllReduce issues immediately after the chains complete
    gk_sb = late.tile([128, 16], F32)
    nc.scalar.copy(gk_sb[:], gk_ps[:, 0:16])
    nc.vector.tensor_tensor(gk_sb[:], gk_sb[:], gk_ps[:, 16:32], op=ALU.add)
    nc.sync.dma_start(ar_in_gk[:], gk_sb[:])
    if n_cores > 1:
        nc.gpsimd.collective_compute(
            "AllReduce", ALU.add,
            replica_groups=[list(range(n_cores))],
            ins=[ar_in_gk.opt()], outs=[ar_out_gk.opt()])
    else:
        nc.sync.dma_start(ar_out_gk[:], ar_in_gk[:])
    arg = late.tile([128, 16], F32)
    nc.sync.dma_start(arg[:], ar_out_gk[:])

    # in-place norm scaling, Fk first so the mm matmuls can start early
    for t in list(range(NT)) + [NT + t for t in range(NT)]:
        if t < NT:
            F_t = Fk[:, t * DIM:(t + 1) * DIM]
            nb = kninv[:, t * 8:(t + 1) * 8].unsqueeze(2) \
                .broadcast_to([128, 8, 128])
        else:
            t2 = t - NT
            F_t = Fq[:, t2 * DIM:(t2 + 1) * DIM]
            nb = qninv[:, t2 * 8:(t2 + 1) * 8].unsqueeze(2) \
                .broadcast_to([128, 8, 128])
        nc.vector.tensor_tensor(F_t.rearrange("p (h d) -> p h d", h=8),
                                F_t.rearrange("p (h d) -> p h d", h=8),
                                nb, op=ALU.mult)

    # ======== Phase 2b: norm scaling, feat_corr, AR payload ========
    nc.scalar.dma_start(identf[:], identf_d[:])
    nc.scalar.dma_start(onesf[:], onesf_d[:])
    nc.scalar.dma_start(ones8f[:], onesf_d[0:1, :])
    nc.scalar.dma_start(mask_nd[:], mask_d[:])
    nc.scalar.dma_start(wp1T[:], wp1T_d[:])
    nc.scalar.dma_start(wp2T[:], wp2T_d[:])
    nc.scalar.dma_start(b1row[:], b1_d[:])
    nc.scalar.dma_start(gbc[:], gbc_d[:])
    nc.scalar.dma_start(bbc[:], bbc_d[:])
    nc.scalar.dma_start(b2bc[:], b2bc_d[:])
    nc.scalar.dma_start(bout[:], bout_d[:])
    nc.gpsimd.partition_broadcast(ybias[:], bout[:])
    for s in range(8):
        nc.scalar.dma_start(WoT[:, s * DIM:(s + 1) * DIM],
                            WoT_d[:, s * DIM:(s + 1) * DIM])

    with tc.tile_pool(name="ph2", bufs=2) as ph2:
        # feat_corr partials, 4 heads per PSUM bank tile -> ar_in_fc
        for g in range(2):
            fc_ps = ps_fc.tile([128, 512], F32, tag="fc", name="fc_ps")
            for hh in range(4):
                h = g * 4 + hh
                for t in range(NT):
                    qsl = qc_tiles[t][:, h * 128:(h + 1) * 128]
                    nc.tensor.matmul(fc_ps[:, hh * 128:(hh + 1) * 128],
                                     qsl, qsl, start=(t == 0),
                                     stop=(t == NT - 1),
                                     skip_group_check=True)
            fc_sb = ph2.tile([128, 512], F32, tag="fcsb", name="fc_sb")
            nc.scalar.copy(fc_sb[:], fc_ps[:])
            nc.sync.dma_start(ar_in_fc[:, g * 512:(g + 1) * 512], fc_sb[:])
    if n_cores > 1:
        nc.gpsimd.collective_compute(
            "AllReduce", ALU.add,
            replica_groups=[list(range(n_cores))],
            ins=[ar_in_fc.opt()], outs=[ar_out_fc.opt()])
    else:
        nc.sync.dma_start(ar_out_fc[:], ar_in_fc[:])
    ar = late.tile([128, H * 128], F32)
    nc.sync.dma_start(ar[:], ar_out_fc[:])

    # -- phase 3 part A (gk-dependent): weight-predictor input + h1 --
    featsq = stat1.tile([128, 8], F32)
    nc.vector.tensor_scalar_mul(featsq[:], arg[:, 0:8], 1.0 / TOK_ALL)
    featsk = stat1.tile([128, 8], F32)
    nc.vector.tensor_scalar_mul(featsk[:], arg[:, 8:16], 1.0 / TOK_ALL)
    nc.tensor.matmul(h1_ap, featsq[:], wp1T[:, 0:128], start=True,
                     stop=False, skip_group_check=True)
    nc.tensor.matmul(h1_ap, featsk[:], wp1T[:, 128:256], start=False,
                     stop=False, skip_group_check=True)
    nc.tensor.matmul(h1_ap, ones8f[:], b1row[:], start=False, stop=True,
                     skip_group_check=True)
    h1 = stat1.tile([8, 128], F32)
    nc.scalar.copy(h1[:], h1_ap)

    # ======== Phase 4a: AR-independent PE work (covers AR flight) ========
    # mm/mv packed 4 heads per PSUM bank; raw (unscaled) eviction to SBUF.
    attn = ctx.enter_context(tc.tile_pool(name="attn", bufs=1))
    mm_raw = {}
    mv_raw = {}
    for j in range(NTASK):
        for g in range(2):
            mm_ps = ps_fc.tile([128, 512], F32, tag="fc", name="mm_ps")
            mv_ps = ps_mv.tile([1, 512], F32, tag="mv", name="mv_ps")
            for hh in range(4):
                h = g * 4 + hh
                for ti in range(4):
                    t = 4 * j + ti
                    sl = slice(t * DIM + h * 128, t * DIM + h * 128 + 128)
                    nc.tensor.matmul(mm_ps[:, hh * 128:(hh + 1) * 128],
                                     Fk[:, sl], Fv[:, sl],
                                     start=(ti == 0), stop=(ti == 3),
                                     skip_group_check=True)
                    nc.tensor.matmul(mv_ps[:, hh * 128:(hh + 1) * 128],
                                     krb[:, t * 8 + h:t * 8 + h + 1],
                                     Fv[:, sl], start=(ti == 0),
                                     stop=(ti == 3), skip_group_check=True)
            mm = attn.tile([128, 512], BF16, tag=f"mm{j}{g}", name="mm")
            nc.scalar.copy(mm[:], mm_ps[:])
            mv = attn.tile([1, 512], BF16, tag=f"mv{j}{g}", name="mv")
            nc.scalar.copy(mv[:], mv_ps[:])
            mm_raw[(j, g)] = mm
            mv_raw[(j, g)] = mv

    # -- phase 3 part B: h1 layernorm + relu (DVE/ACT, overlaps fqT) --
    sq3 = late.tile([128, H * 128], BF16)   # write-only square scratch
    w_mu = stat1.tile([8, 4], F32)
    nc.vector.reduce_sum(w_mu[:, 0:1], h1[:], axis=AX.X)
    nc.vector.tensor_scalar_mul(w_mu[:, 0:1], w_mu[:, 0:1], 1.0 / D)
    nc.scalar.activation(sq3[0:8, 0:128], h1[:], AF.Square,
                         accum_out=w_mu[:, 1:2])
    nc.vector.tensor_scalar_mul(w_mu[:, 1:2], w_mu[:, 1:2], 1.0 / D)
    nc.vector.tensor_tensor(w_mu[:, 2:3], w_mu[:, 0:1], w_mu[:, 0:1],
                            op=ALU.mult)
    nc.vector.tensor_tensor(w_mu[:, 2:3], w_mu[:, 1:2], w_mu[:, 2:3],
                            op=ALU.subtract)
    nc.scalar.activation(w_mu[:, 3:4], w_mu[:, 2:3], AF.Sqrt, bias=eps[0:8, :])
    nc.vector.reciprocal(w_mu[:, 3:4], w_mu[:, 3:4])
    h1n = stat1.tile([8, 128], F32)
    nc.vector.tensor_scalar(h1n[:], h1[:], w_mu[:, 0:1], w_mu[:, 3:4],
                            ALU.subtract, ALU.mult)
    nc.vector.tensor_tensor(h1n[:], h1n[:], gbc[:], op=ALU.mult)
    nc.vector.tensor_tensor(h1n[:], h1n[:], bbc[:], op=ALU.add)
    nc.vector.tensor_scalar_max(h1n[:], h1n[:], 0.0)

    # Fq^T tiles (scaled Fq, bf16): 8 transposes (2 head-groups) per PSUM
    # bank tile + 1 wide copy
    fqTs = {}
    for j in range(NTASK):
        for g in range(4):
            tr_ps = ps_tr.tile([128, 1024], BF16, tag="tr", name="tr_ps")
            for hh in range(2):
                h = g * 2 + hh
                for ti in range(4):
                    t = 4 * j + ti
                    sl = slice(t * DIM + h * 128, t * DIM + h * 128 + 128)
                    nc.tensor.transpose(
                        tr_ps[:, hh * 512 + ti * 128:hh * 512 + ti * 128 + 128],
                        Fq[:, sl], identb[:])
            fqT = attn.tile([128, 1024], BF16, tag=f"fqT{j}{g}", name="fqT")
            if g % 2 == 0:
                nc.scalar.copy(fqT[:], tr_ps[:])
            else:
                nc.vector.tensor_copy(fqT[:], tr_ps[:])
            fqTs[(j, g)] = fqT

    # -- phase 3 part C: second predictor layer + softmax --
    nc.tensor.matmul(h1T_ap, h1n[:], identf[:], is_transpose=True,
                     skip_group_check=True)
    h1T = stat1.tile([128, 8], F32)
    nc.scalar.copy(h1T[:], h1T_ap)
    nc.tensor.matmul(lg_ap, h1T[:], wp2T[:], start=True, stop=True,
                     skip_group_check=True)
    lg = stat1.tile([8, 8], F32)
    nc.scalar.copy(lg[:, 0:3], lg_ap)
    nc.vector.tensor_tensor(lg[:, 0:3], lg[:, 0:3], b2bc[:], op=ALU.add)
    # logits are O(1): skip the (mathematically redundant) max-subtraction
    nc.scalar.activation(lg[:, 0:3], lg[:, 0:3], AF.Exp)
    nc.vector.reduce_sum(lg[:, 4:5], lg[:, 0:3], axis=AX.X)
    nc.vector.reciprocal(lg[:, 4:5], lg[:, 4:5])
    nc.vector.tensor_scalar(lg[:, 0:3], lg[:, 0:3], lg[:, 4:5], None, ALU.mult)

    # q_ratio rows: one whole-tile transpose with (j,h,ti)-reordered rows,
    # then contiguous-partition flatten DMAs (partition-strided DMA slices
    # are not supported)
    nc.tensor.matmul(qrT_ap, qrb[:], identb[:], is_transpose=True,
                     skip_group_check=True)
    qrT_sb = stat1.tile([64, 128], BF16)
    nc.vector.tensor_copy(qrT_sb[:], qrT_ap)
    # q_ratio rows flattened pre-AR (DMAs fly during the collective)
    wqrs = {}
    for j in range(NTASK):
        for h in range(H):
            r0 = j * 32 + h * 4
            wqr = attn.tile([1, 512], BF16, tag=f"wqr{h}{j}", name="wqr")
            nc.sync.dma_start(wqr[:], qrT_sb[r0:r0 + 4, :])
            wqrs[(h, j)] = wqr

    # -- phase 3 part D (fc-dependent): decorrelation scale --
    ssq = stat1.tile([128, 8], F32)
    msk = late.tile([128, H * 128], F32)
    nc.vector.tensor_tensor(msk[:], ar[:], mask_nd[:], op=ALU.mult)
    nc.scalar.activation(sq3[:], msk[:], AF.Square, scale=1.0 / TOK_ALL)
    nc.vector.reduce_sum(ssq[:],
                         sq3[:].rearrange("p (h d) -> p h d", h=8),
                         axis=AX.X)
    nc.tensor.matmul(ss_ap, ssq[:], onesf[:], start=True, stop=True,
                     skip_group_check=True)
    dsc = stat1.tile([8, 8], F32)
    nc.scalar.activation(dsc[:, 0:1], ss_ap[0:8, 0:1], AF.Sqrt)
    nc.scalar.activation(dsc[:, 1:2], dsc[:, 0:1], AF.Exp, scale=-5.0 / (D * D))
    # alpha = w0 + w1*dsc ; ww = w2 ; broadcast to 128 partitions
    aw = stat1.tile([8, 2], F32)
    nc.vector.tensor_tensor(aw[:, 0:1], lg[:, 1:2], dsc[:, 1:2], op=ALU.mult)
    nc.vector.tensor_tensor(aw[:, 0:1], aw[:, 0:1], lg[:, 0:1], op=ALU.add)
    nc.vector.tensor_copy(aw[:, 1:2], lg[:, 2:3])
    nc.tensor.matmul(awTa_ap, aw[:, 0:1], identf[:],
                     is_transpose=True, skip_group_check=True)
    nc.tensor.matmul(awTw_ap, aw[:, 1:2], identf[:],
                     is_transpose=True, skip_group_check=True)
    awTa = stat1.tile([1, 8], F32)
    nc.scalar.copy(awTa[:], awTa_ap)
    awTw = stat1.tile([1, 8], F32)
    nc.scalar.copy(awTw[:], awTw_ap)
    abc = stat1.tile([128, 8], F32)
    nc.gpsimd.partition_broadcast(abc[:], awTa[:])
    wbc = stat1.tile([128, 8], F32)
    nc.gpsimd.partition_broadcast(wbc[:], awTw[:])

    # ======== Phase 4b + 5: scaled attention + output projection ========
    with tc.tile_pool(name="ph4", bufs=2) as ph4, \
         tc.tile_pool(name="o1pool", bufs=1) as o1pool:
        o1_tiles = {}
        mm_sb = {}
        mv_sb = {}
        for j in range(NTASK):
            # scale mm/mv by the dynamic per-head weights (batched, 4 heads)
            for g in range(2):
                msb = ph4.tile([128, 512], BF16, tag=f"mmsb{j}{g}",
                               name="mm_sb")
                ab = abc[:, g * 4:(g + 1) * 4].unsqueeze(2) \
                    .broadcast_to([128, 4, 128])
                nc.vector.tensor_tensor(
                    msb[:].rearrange("p (h d) -> p h d", h=4),
                    mm_raw[(j, g)][:].rearrange("p (h d) -> p h d", h=4),
                    ab, op=ALU.mult)
                mm_sb[(j, g)] = msb
                vsb = ph4.tile([1, 512], BF16, tag=f"mvsb{j}{g}",
                               name="mv_sb")
                wb = wbc[0:1, g * 4:(g + 1) * 4].unsqueeze(2) \
                    .broadcast_to([1, 4, 128])
                nc.vector.tensor_tensor(
                    vsb[:].rearrange("p (h d) -> p h d", h=4),
                    mv_raw[(j, g)][:].rearrange("p (h d) -> p h d", h=4),
                    wb, op=ALU.mult)
                mv_sb[(j, g)] = vsb
        for j in range(NTASK):
            for h in range(H):
                g, hh = h // 4, h % 4
                wqr = wqrs[(h, j)]
                opool = ps_o1 if h % 2 == 0 else ps_proj
                o1_ps = opool.tile([128, 512], F32,
                                   tag="o1" if h % 2 == 0 else "proj",
                                   name="o1_ps")
                nc.tensor.matmul(
                    o1_ps[:], mm_sb[(j, g)][:, hh * 128:(hh + 1) * 128],
                    fqTs[(j, h // 2)][:, (h % 2) * 512:(h % 2) * 512 + 512],
                    start=True, stop=False)
                nc.tensor.matmul(o1_ps[:],
                                 mv_sb[(j, g)][:, hh * 128:(hh + 1) * 128],
                                 wqr[:], start=False, stop=True)
                o1 = o1pool.tile([128, 512], BF16, tag=f"o1sb{h}{j}",
                                 name="o1_sb")
                if h % 2 == 0:
                    nc.scalar.copy(o1[:], o1_ps[:])
                else:
                    nc.vector.tensor_copy(o1[:], o1_ps[:])
                o1_tiles[(h, j)] = o1

        # ---- output projection, both tasks ----
        for j in range(NTASK):
            for t in range(4 * j, 4 * j + 4):
                ti = t % 4
                for half in range(2):
                    o = half * 512
                    opool2 = ps_proj if half == 0 else ps_o1
                    op_ps = opool2.tile([128, 512], F32,
                                        tag="proj" if half == 0 else "o1",
                                        name="op_ps")
                    for h in range(H):
                        nc.tensor.matmul(
                            op_ps[:],
                            o1_tiles[(h, j)][:, ti * 128:(ti + 1) * 128],
                            WoT[:, h * DIM + o: h * DIM + o + 512],
                            start=(h == 0), stop=(h == H - 1))
                    ysb = ph4.tile([128, 512], BF16, tag="ysb", name="ysb")
                    nc.vector.tensor_tensor(ysb[:], op_ps[:],
                                            ybias[:, o:o + 512], op=ALU.add)
                    nc.sync.dma_start(y[t * 128:(t + 1) * 128, o:o + 512],
                                      ysb[:])


_BUILT = {}


def _build(n_cores=N_CORES):
    if n_cores in _BUILT:
        return _BUILT[n_cores]
    nc = bacc.Bacc("TRN2", target_bir_lowering=False, debug=False,
                   num_devices=n_cores)
    in_specs = [
        ("xn_q", [T, DIM], BF16), ("xn_k", [T, DIM], BF16),
        ("xn_v", [T, DIM], BF16),
        ("xT_q", [128, NT * DIM], BF16), ("xT_k", [128, NT * DIM], BF16),
        ("xT_v", [128, NT * DIM], BF16),
        ("Wp", [128, 8 * DIM], BF16), ("WoT", [128, 8 * DIM], BF16),
        ("nv2", [2, DIM], BF16), ("bout", [1, DIM], F32),
        ("onesb", [128, 8], BF16), ("onesf", [128, 8], F32),
        ("identb", [128, 128], BF16), ("identf", [8, 8], F32),
        ("mask", [128, 1024], F32),
        ("wp1T", [128, 256], F32), ("wp2T", [128, 3], F32),
        ("b1row", [1, 128], F32),
        ("gbc", [8, 128], F32), ("bbc", [8, 128], F32), ("b2bc", [8, 3], F32),
    ]
    in_aps = [nc.dram_tensor(n, s, d, kind="ExternalInput").ap()
              for n, s, d in in_specs]
    y_ap = nc.dram_tensor("y", [T, DIM], BF16, kind="ExternalOutput").ap()
    with tile.TileContext(nc) as tc:
        attn_kernel(tc, [y_ap], in_aps, n_cores=n_cores)
    nc.compile()
    _BUILT[n_cores] = nc
    return nc


def kernel(q, k, v, ln_g, ln_b, w_in, wp_w1, wp_b1, wp_ln_g, wp_ln_b,
           wp_w2, wp_b2, w_out, b_out):
    q = np.asarray(q, dtype=np.float32)
    k = np.asarray(k, dtype=np.float32)
    v = np.asarray(v, dtype=np.float32)
    ln_g = np.asarray(ln_g, np.float32); ln_b = np.asarray(ln_b, np.float32)
    w_in = np.asarray(w_in, np.float32); w_out = np.asarray(w_out, np.float32)
    b_out = np.asarray(b_out, np.float32)
    wp_w1 = np.asarray(wp_w1, np.float32); wp_b1 = np.asarray(wp_b1, np.float32)
    wp_ln_g = np.asarray(wp_ln_g, np.float32)
    wp_ln_b = np.asarray(wp_ln_b, np.float32)
    wp_w2 = np.asarray(wp_w2, np.float32); wp_b2 = np.asarray(wp_b2, np.float32)

    bf = ml_dtypes.bfloat16

    # host weight prep (folded layernorm)
    W = w_in.T                                     # [DIM, HD]
    Wp = (ln_g[:, None] * W)
    negu = -(ln_g @ W)
    vrow = (ln_b @ W)
    nv2 = np.stack([negu, vrow], axis=0)           # [2, DIM]
    Wp_t = np.ascontiguousarray(
        Wp.reshape(8, 128, DIM).transpose(1, 0, 2)).reshape(128, -1)
    WoT = np.ascontiguousarray(
        w_out.T.reshape(8, 128, DIM).transpose(1, 0, 2)).reshape(128, -1)
    shared = {
        "Wp": Wp_t.astype(bf), "WoT": WoT.astype(bf),
        "nv2": nv2.astype(bf),
        "bout": np.ascontiguousarray(b_out[None, :], np.float32),
        "onesb": np.ones((128, 8), bf),
        "onesf": np.ones((128, 8), np.float32),
        "identb": np.eye(128).astype(bf),
        "identf": np.eye(8, dtype=np.float32),
        "mask": np.ascontiguousarray(
            np.tile((1.0 - np.eye(128)).astype(np.float32), (1, 8))),
        "wp1T": np.ascontiguousarray(wp_w1.T.reshape(2, 128, 128)
                                     .transpose(1, 0, 2)).reshape(128, 256),
        "wp2T": np.ascontiguousarray(wp_w2.T),
        "b1row": np.ascontiguousarray(wp_b1[None, :]),
        "gbc": np.ascontiguousarray(np.tile(wp_ln_g[None, :], (8, 1))),
        "bbc": np.ascontiguousarray(np.tile(wp_ln_b[None, :], (8, 1))),
        "b2bc": np.ascontiguousarray(np.tile(wp_b2[None, :], (8, 1))),
    }

    qf = q.reshape(QB * N, DIM)
    kf = k.reshape(QB * N, DIM)
    vf = v.reshape(QB * N, DIM)
    in_maps = []
    for c in range(N_CORES):
        sl = slice(c * T, (c + 1) * T)
        m = dict(shared)
        for nm, arr in (("q", qf[sl]), ("k", kf[sl]), ("v", vf[sl])):
            ab = arr.astype(bf)
            m[f"xn_{nm}"] = np.ascontiguousarray(ab)
            m[f"xT_{nm}"] = np.ascontiguousarray(
                ab.reshape(NT, 128, 8, 128).transpose(3, 0, 2, 1)
            ).reshape(128, NT * DIM)
        in_maps.append(m)

    nc = _build()
    res = bass_utils.run_bass_kernel_spmd(nc, in_maps,
                                          core_ids=list(range(N_CORES)))
    global LAST_RESULTS
    LAST_RESULTS = res
    out = np.concatenate([np.asarray(r["y"], np.float32)
                          for r in res.results], axis=0)
    return out.reshape(QB, N, DIM)


LAST_RESULTS = None


# revision 36
# speedup vs baseline: 1.4257x; 1.0327x over previous
"""Trainium2 Bass kernel for nn_Attention_9096740733536 (sparse_attention).

Sharding: data-parallel over the QB (task) dim across 8 cores (2 tasks/core),
one mid-kernel AllReduce of [feat_corr partials | q_global | k_global] sums.
The attention math is algebraically collapsed: mixed scores are linear (no
softmax), so
  out[h,q] = alpha_h*(Fq/qn) @ ((Fk/kn)^T @ Fv) + ww_h*q_ratio (x) (kr^T Fv)
with 128x128 inner matrices instead of 512x512 score matrices, and layernorm
is folded into the input projection via a merged contraction-2 PSUM rank-1.
All heavy matmuls/transposes run in bf16 (1 cyc/row on PE for any width),
inputs/weights are staged in bf16 on the host (halves HBM traffic), and the
elementwise work is spread across ACT/DVE with wide batched instructions.
"""
import numpy as np
import ml_dtypes
from contextlib import ExitStack

import concourse.bass as bass
import concourse.tile as tile
from concourse import bacc, mybir
from concourse import bass_utils
from concourse._compat import with_exitstack

F32 = mybir.dt.float32
BF16 = mybir.dt.bfloat16
AF = mybir.ActivationFunctionType
ALU = mybir.AluOpType
AX = mybir.AxisListType

H, D, DIM = 8, 128, 1024
QB, N = 16, 512
N_CORES = 8
T = QB * N // N_CORES          # 1024 tokens per core
NT = T // 128                  # 8 token tiles per core
NTASK = T // N                 # 2 tasks per core
LN_EPS = 1e-5
TOK_ALL = float(QB * N)


@with_exitstack
def attn_kernel(ctx: ExitStack, tc: tile.TileContext, outs, ins, n_cores=N_CORES):
    nc = tc.nc
    y = outs[0]
    (xn_q, xn_k, xn_v, xT_q, xT_k, xT_v, Wp_d, WoT_d, nv2_d, bout_d,
     onesb_d, onesf_d, identb_d, identf_d, mask_d, wp1T_d, wp2T_d, b1_d,
     gbc_d, bbc_d, b2bc_d) = ins

    consts = ctx.enter_context(tc.tile_pool(name="consts", bufs=1))
    fpool = ctx.enter_context(tc.tile_pool(name="fpool", bufs=1))
    stat1 = ctx.enter_context(tc.tile_pool(name="stat1", bufs=1))
    dram = ctx.enter_context(tc.tile_pool(name="dram", bufs=1, space="DRAM"))

    ps_proj = ctx.enter_context(tc.tile_pool(name="ps_proj", bufs=2, space="PSUM"))
    ps_fc = ctx.enter_context(tc.tile_pool(name="ps_fc", bufs=1, space="PSUM"))
    ps_trmv = ctx.enter_context(tc.tile_pool(name="ps_trmv", bufs=2,
                                             space="PSUM"))
    ps_o1 = ctx.enter_context(tc.tile_pool(name="ps_o1", bufs=2, space="PSUM"))
    ps_sm = ctx.enter_context(tc.tile_pool(name="ps_sm", bufs=1, space="PSUM"))

    # one shared PSUM bank for all small accumulators, carved manually
    psc = ps_sm.tile([128, 512], F32, tag="sm")
    trp_aps = [psc[0:2, i * 64:(i + 1) * 64].bitcast(BF16) for i in range(3)]
    qrT_ap = psc[0:64, 192:256].bitcast(BF16)       # [64, 128] bf16
    gk_ap = psc[:, 256:288]                          # [128, 32] f32 (per-task)
    ss_ap = psc[0:8, 288:296]                        # [8, 8]
    h1_ap = psc[0:8, 296:424]                        # [8, 128]
    h1T_ap = psc[:, 424:432]                         # [128, 8]
    lg_ap = psc[0:8, 432:435]                        # [8, 3]
    awTa_ap = psc[0:1, 435:443]                      # [1, 8]
    awTw_ap = psc[0:1, 443:451]                      # [1, 8]

    # ---- small constants (long-lived) ----
    identb = consts.tile([128, 128], BF16)
    nc.scalar.dma_start(identb[:], identb_d[:])
    onesb = consts.tile([128, 8], BF16)
    nc.scalar.dma_start(onesb[:], onesb_d[:])
    nv2 = consts.tile([2, DIM], BF16)
    nc.scalar.dma_start(nv2[:], nv2_d[:])
    # phase-3-only constants are DMA'd later (phase 2b) to keep the ACT
    # queue clear during startup
    identf = consts.tile([8, 8], F32)
    onesf = consts.tile([128, 8], F32)
    ones8f = consts.tile([1, 8], F32)
    mask_nd = consts.tile([128, H * 128], F32)
    wp1T = consts.tile([128, 256], F32)
    wp2T = consts.tile([128, 3], F32)
    b1row = consts.tile([1, 128], F32)
    gbc = consts.tile([8, 128], F32)
    bbc = consts.tile([8, 128], F32)
    b2bc = consts.tile([8, 3], F32)
    ybias = consts.tile([128, DIM], F32)
    bout = consts.tile([1, DIM], F32)
    eps = consts.tile([128, 1], F32)
    nc.vector.memset(eps[:], LN_EPS)

    # ---- persistent F tensors: [128 tok, t*1024 + h*128 + d] bf16 ----
    late = ctx.enter_context(tc.tile_pool(name="late", bufs=1))
    WoT = late.tile([128, 8 * DIM], BF16)
    Fq = fpool.tile([128, NT * DIM], BF16)
    Fk = fpool.tile([128, NT * DIM], BF16)
    Fv = fpool.tile([128, NT * DIM], BF16)

    xns = [xn_q, xn_k, xn_v]
    xTs = [xT_q, xT_k, xT_v]
    Fs = [Fq, Fk, Fv]

    # ======== Phase 1: folded-LN projection + interleaved F stats ========
    qss = stat1.tile([128, 64], F32)   # col t*8+h : sumsq over d of Fq
    qsm = stat1.tile([128, 64], F32)   # sums over d
    kss = stat1.tile([128, 64], F32)
    ksm = stat1.tile([128, 64], F32)
    qmean = stat1.tile([128, 64], F32)
    qninv = stat1.tile([128, 64], F32)
    kninv = stat1.tile([128, 64], F32)
    qr = stat1.tile([128, 64], F32)
    kr = stat1.tile([128, 64], F32)
    qrb = stat1.tile([128, 64], BF16)
    krb = stat1.tile([128, 64], BF16)
    rscr = stat1.tile([128, 96], F32)  # ratio-chain scratch (3x32 per half)
    ar_in_fc = dram.tile([128, H * 128], F32)
    ar_out_fc = dram.tile([128, H * 128], F32)
    ar_in_gk = dram.tile([128, 16], F32)
    ar_out_gk = dram.tile([128, 16], F32)
    gk_ps = gk_ap
    qcpool = ctx.enter_context(tc.tile_pool(name="qcpool", bufs=1))
    qc_tiles = {}

    def derived(ss, sm, ninv, ratio, s):
        # ninv = 1/sqrt(ss); var = ss/127 - sm^2/(128*127)
        # ratio = 2*min(var,1)/(var+1)
        w = s.stop - s.start
        nc.scalar.activation(ninv[:, s], ss[:, s], AF.Sqrt)
        nc.vector.reciprocal(ninv[:, s], ninv[:, s])
        t1 = rscr[:, 0:w]
        nc.vector.tensor_tensor(t1, sm[:, s], sm[:, s], op=ALU.mult)
        nc.vector.tensor_scalar_mul(t1, t1, 1.0 / (D * (D - 1)))
        t2 = rscr[:, w:2 * w]
        nc.vector.tensor_scalar_mul(t2, ss[:, s], 1.0 / (D - 1))
        var = rscr[:, 2 * w:3 * w]
        nc.vector.tensor_tensor(var, t2, t1, op=ALU.subtract)
        nc.vector.tensor_scalar(t1, var, 1.0, 2.0, ALU.min, ALU.mult)
        nc.vector.tensor_scalar_add(t2, var, 1.0)
        nc.vector.reciprocal(t2, t2)
        nc.vector.tensor_tensor(ratio[:, s], t1, t2, op=ALU.mult)

    with tc.tile_pool(name="ph1", bufs=1) as ph1, \
         tc.tile_pool(name="xpool", bufs=3) as xpool, \
         tc.tile_pool(name="sqpool", bufs=2) as sqpool, \
         tc.tile_pool(name="spool", bufs=3) as spool:
        Wp = ph1.tile([128, 8 * DIM], BF16)
        xT_0 = xpool.tile([128, DIM], BF16, tag="xT")
        nc.gpsimd.dma_start(xT_0[:], xTs[0][:, 0:DIM])
        for s in range(8):
            eng = nc.gpsimd if s % 2 == 0 else nc.scalar
            eng.dma_start(Wp[:, s * DIM:(s + 1) * DIM],
                          Wp_d[:, s * DIM:(s + 1) * DIM])
        warm = {}
        for half in range(2):
            o = half * 512
            acc = ps_proj.tile([128, 512], F32, tag="proj")
            for s in range(8):
                nc.tensor.matmul(acc[:], xT_0[:, s * 128:(s + 1) * 128],
                                 Wp[:, s * DIM + o: s * DIM + o + 512],
                                 start=(s == 0), stop=False,
                                 skip_group_check=True)
            warm[half] = acc
        def scale_tile(which, tt):
            if which == "k":
                F_t = Fk[:, tt * DIM:(tt + 1) * DIM]
                nb = kninv[:, tt * 8:(tt + 1) * 8].unsqueeze(2) \
                    .broadcast_to([128, 8, 128])
            else:
                F_t = Fq[:, tt * DIM:(tt + 1) * DIM]
                nb = qninv[:, tt * 8:(tt + 1) * 8].unsqueeze(2) \
                    .broadcast_to([128, 8, 128])
            nc.vector.tensor_tensor(F_t.rearrange("p (h d) -> p h d", h=8),
                                    F_t.rearrange("p (h d) -> p h d", h=8),
                                    nb, op=ALU.mult)

        SPREAD = {5: [("k", 0), ("k", 1)], 6: [("k", 2), ("k", 3), ("q", 0)],
                  7: [("q", 1), ("q", 2), ("q", 3)]}
        for t in range(NT):
            st = spool.tile([128, 12], F32, tag="st")
            bn6 = spool.tile([128, 36], F32, tag="bn6")
            rsig = spool.tile([128, 3], F32, tag="rsig")
            for i in range(3):
                xn = xpool.tile([128, DIM], BF16, tag="xn")
                nc.sync.dma_start(xn[:], xns[i][t * 128:(t + 1) * 128, :])
                nc.vector.bn_stats(bn6[:, i * 12:i * 12 + 6], xn[:, 0:512])
                nc.vector.bn_stats(bn6[:, i * 12 + 6:i * 12 + 12],
                                   xn[:, 512:1024])
                nc.vector.bn_aggr(st[:, 2 * i:2 * i + 2],
                                  bn6[:, i * 12:i * 12 + 12])
            # interleave (mu_i, sig_i) into cols 6..12
            nc.vector.tensor_copy(st[:, 6:12:2], st[:, 0:6:2])
            nc.scalar.activation(st[:, 7:12:2], st[:, 1:6:2], AF.Sqrt,
                                 bias=eps[:])
            nc.vector.reciprocal(rsig[:], st[:, 7:12:2])
            stb = spool.tile([128, 6], BF16, tag="stb")
            nc.vector.tensor_copy(stb[:], st[:, 6:12])
            for i in range(3):
                nc.tensor.matmul(trp_aps[i], stb[:, 2 * i:2 * i + 2],
                                 identb[:], is_transpose=True,
                                 skip_group_check=True)
            rows_all = spool.tile([2, 384], BF16, tag="rows", name="rows")
            nc.vector.tensor_copy(rows_all[:],
                                  psc[0:2, 0:192].bitcast(BF16))
            rows = [rows_all[:, i * 128:(i + 1) * 128] for i in range(3)]
            for i in range(3):
                if t == 0 and i == 0:
                    xT_t = xT_0
                else:
                    xT_t = xpool.tile([128, DIM], BF16, tag="xT")
                    nc.gpsimd.dma_start(xT_t[:],
                                        xTs[i][:, t * DIM:(t + 1) * DIM])
                for half in range(2):
                    o = half * 512
                    if t == 0 and i == 0:
                        acc = warm[half]
                    else:
                        acc = ps_proj.tile([128, 512], F32, tag="proj")
                        for s in range(8):
                            nc.tensor.matmul(
                                acc[:], xT_t[:, s * 128:(s + 1) * 128],
                                Wp[:, s * DIM + o: s * DIM + o + 512],
                                start=(s == 0), stop=False)
                    nc.tensor.matmul(acc[:], rows[i], nv2[:, o:o + 512],
                                     start=False, stop=True,
                                     skip_group_check=(t == 0 and i == 0))
                    dst = Fs[i][:, t * DIM + o: t * DIM + o + 512]
                    nc.scalar.activation(dst, acc[:], AF.Identity,
                                         scale=rsig[:, i:i + 1])
            # ---- interleaved per-tile F stats (run on DVE/ACT slack) ----
            Fq_t = Fq[:, t * DIM:(t + 1) * DIM]
            Fk_t = Fk[:, t * DIM:(t + 1) * DIM]
            c8 = slice(t * 8, (t + 1) * 8)
            nc.vector.reduce_sum(qsm[:, c8],
                                 Fq_t.rearrange("p (h d) -> p h d", h=8),
                                 axis=AX.X)
            nc.vector.reduce_sum(ksm[:, c8],
                                 Fk_t.rearrange("p (h d) -> p h d", h=8),
                                 axis=AX.X)
            sq = sqpool.tile([128, DIM], BF16, tag="sq")
            nc.scalar.activation(sq[:], Fq_t, AF.Square)
            nc.vector.reduce_sum(qss[:, c8],
                                 sq[:].rearrange("p (h d) -> p h d", h=8),
                                 axis=AX.X)
            sk = sqpool.tile([128, DIM], BF16, tag="sk")
            nc.scalar.activation(sk[:], Fk_t, AF.Square)
            nc.vector.reduce_sum(kss[:, c8],
                                 sk[:].rearrange("p (h d) -> p h d", h=8),
                                 axis=AX.X)
            for wq, tt in SPREAD.get(t, []):
                scale_tile(wq, tt)
            # ---- per-task ratio/norm chains + qc + gk once available ----
            if t % 4 == 3:
                jh = t // 4
                s4 = slice(jh * 32, jh * 32 + 32)
                nc.vector.tensor_scalar_mul(qmean[:, s4], qsm[:, s4], 1.0 / D)
                derived(qss, qsm, qninv, qr, s4)
                derived(kss, ksm, kninv, kr, s4)
                # store qrb with (h, ti)-permuted columns so the qrT
                # transpose yields contiguous (j, h, ti) rows
                nc.vector.tensor_copy(
                    qrb[:, s4].rearrange("p (h ti) -> p h ti", h=8, ti=4),
                    qr[:, s4].rearrange("p (ti h) -> p ti h", ti=4,
                                        h=8).transpose([0, 2, 1]))
                nc.vector.tensor_copy(krb[:, s4], kr[:, s4])
                for tt in range(4 * jh, 4 * jh + 4):
                    Fq_tt = Fq[:, tt * DIM:(tt + 1) * DIM]
                    qc = qcpool.tile([128, DIM], BF16, tag=f"qc{tt}",
                                     name="qc")
                    qmb = qmean[:, tt * 8:(tt + 1) * 8].unsqueeze(2) \
                        .broadcast_to([128, 8, 128])
                    nc.vector.tensor_tensor(
                        qc[:].rearrange("p (h d) -> p h d", h=8),
                        Fq_tt.rearrange("p (h d) -> p h d", h=8),
                        qmb, op=ALU.subtract)
                    qc_tiles[tt] = qc
                if jh == 0:
                    # task-0 global-sum chains while projection continues
                    for h in range(H):
                        for tt in range(4):
                            sl = slice(tt * DIM + h * 128,
                                       tt * DIM + h * 128 + 128)
                            nc.tensor.matmul(gk_ps[:, h:h + 1], Fq[:, sl],
                                             onesb[:, 0:1],
                                             start=(tt == 0), stop=(tt == 3),
                                             skip_group_check=True)
                        for tt in range(4):
                            sl = slice(tt * DIM + h * 128,
                                       tt * DIM + h * 128 + 128)
                            nc.tensor.matmul(gk_ps[:, 8 + h:9 + h],
                                             Fq[:, sl] if False else Fk[:, sl],
                                             onesb[:, 0:1],
                                             start=(tt == 0), stop=(tt == 3),
                                             skip_group_check=True)

    # task-1 global-sum chains
    for h in range(H):
        for tt in range(4, 8):
            sl = slice(tt * DIM + h * 128, tt * DIM + h * 128 + 128)
            nc.tensor.matmul(gk_ps[:, 16 + h:16 + h + 1], Fq[:, sl],
                             onesb[:, 0:1], start=(tt == 4), stop=(tt == 7),
                             skip_group_check=True)
        for tt in range(4, 8):
            sl = slice(tt * DIM + h * 128, tt * DIM + h * 128 + 128)
            nc.tensor.matmul(gk_ps[:, 24 + h:25 + h], Fk[:, sl],
                             onesb[:, 0:1], start=(tt == 4), stop=(tt == 7),
                             skip_group_check=True)

    # gk reduction hoisted before the scales on the DVE queue so the gk
    # AllReduce issues immediately after the chains complete
    gk_sb = late.tile([128, 16], F32)
    nc.scalar.copy(gk_sb[:], gk_ps[:, 0:16])
    nc.vector.tensor_tensor(gk_sb[:], gk_sb[:], gk_ps[:, 16:32], op=ALU.add)
    nc.sync.dma_start(ar_in_gk[:], gk_sb[:])
    if n_cores > 1:
        nc.gpsimd.collective_compute(
            "AllReduce", ALU.add,
            replica_groups=[list(range(n_cores))],
            ins=[ar_in_gk.opt()], outs=[ar_out_gk.opt()])
    else:
        nc.sync.dma_start(ar_out_gk[:], ar_in_gk[:])
    arg = late.tile([128, 16], F32)
    nc.sync.dma_start(arg[:], ar_out_gk[:])

    # remaining (task-1) norm scales, Fk first so mm j1 starts early
    for wq, tt in [("k", 4), ("k", 5), ("k", 6), ("k", 7),
                   ("q", 4), ("q", 5), ("q", 6), ("q", 7)]:
        scale_tile(wq, tt)

    # ======== Phase 2b: norm scaling, feat_corr, AR payload ========
    nc.scalar.dma_start(identf[:], identf_d[:])
    nc.scalar.dma_start(onesf[:], onesf_d[:])
    nc.scalar.dma_start(ones8f[:], onesf_d[0:1, :])
    nc.scalar.dma_start(mask_nd[:], mask_d[:])
    nc.scalar.dma_start(wp1T[:], wp1T_d[:])
    nc.scalar.dma_start(wp2T[:], wp2T_d[:])
    nc.scalar.dma_start(b1row[:], b1_d[:])
    nc.scalar.dma_start(gbc[:], gbc_d[:])
    nc.scalar.dma_start(bbc[:], bbc_d[:])
    nc.scalar.dma_start(b2bc[:], b2bc_d[:])
    nc.scalar.dma_start(bout[:], bout_d[:])
    nc.gpsimd.partition_broadcast(ybias[:], bout[:])
    for s in range(8):
        nc.scalar.dma_start(WoT[:, s * DIM:(s + 1) * DIM],
                            WoT_d[:, s * DIM:(s + 1) * DIM])

    with tc.tile_pool(name="ph2", bufs=2) as ph2:
        # feat_corr partials, 4 heads per PSUM bank tile -> ar_in_fc
        for g in range(2):
            fc_ps = ps_fc.tile([128, 512], F32, tag="fc", name="fc_ps")
            for hh in range(4):
                h = g * 4 + hh
                for t in range(NT):
                    qsl = qc_tiles[t][:, h * 128:(h + 1) * 128]
                    nc.tensor.matmul(fc_ps[:, hh * 128:(hh + 1) * 128],
                                     qsl, qsl, start=(t == 0),
                                     stop=(t == NT - 1),
                                     skip_group_check=True)
            fc_sb = ph2.tile([128, 512], F32, tag="fcsb", name="fc_sb")
            nc.scalar.copy(fc_sb[:], fc_ps[:])
            nc.sync.dma_start(ar_in_fc[:, g * 512:(g + 1) * 512], fc_sb[:])
    if n_cores > 1:
        nc.gpsimd.collective_compute(
            "AllReduce", ALU.add,
            replica_groups=[list(range(n_cores))],
            ins=[ar_in_fc.opt()], outs=[ar_out_fc.opt()])
    else:
        nc.sync.dma_start(ar_out_fc[:], ar_in_fc[:])
    ar = late.tile([128, H * 128], F32)
    nc.sync.dma_start(ar[:], ar_out_fc[:])

    # -- phase 3 part A (gk-dependent): weight-predictor input + h1 --
    featsq = stat1.tile([128, 8], F32)
    nc.vector.tensor_scalar_mul(featsq[:], arg[:, 0:8], 1.0 / TOK_ALL)
    featsk = stat1.tile([128, 8], F32)
    nc.vector.tensor_scalar_mul(featsk[:], arg[:, 8:16], 1.0 / TOK_ALL)
    nc.tensor.matmul(h1_ap, featsq[:], wp1T[:, 0:128], start=True,
                     stop=False, skip_group_check=True)
    nc.tensor.matmul(h1_ap, featsk[:], wp1T[:, 128:256], start=False,
                     stop=False, skip_group_check=True)
    nc.tensor.matmul(h1_ap, ones8f[:], b1row[:], start=False, stop=True,
                     skip_group_check=True)
    h1 = stat1.tile([8, 128], F32)
    nc.scalar.copy(h1[:], h1_ap)

    # ======== Phase 4a: AR-independent PE work (covers AR flight) ========
    # mm/mv packed 4 heads per PSUM bank; raw (unscaled) eviction to SBUF.
    attn = ctx.enter_context(tc.tile_pool(name="attn", bufs=1))
    mm_raw = {}
    mv_raw = {}
    for j in range(NTASK):
        for g in range(2):
            mm_ps = ps_fc.tile([128, 512], F32, tag="fc", name="mm_ps")
            mv_tile = ps_trmv.tile([128, 1024], BF16, tag="trmv",
                                   name="mv_tile")
            mv_ps = mv_tile[0:1, 0:1024].bitcast(F32)
            for hh in range(4):
                h = g * 4 + hh
                for ti in range(4):
                    t = 4 * j + ti
                    sl = slice(t * DIM + h * 128, t * DIM + h * 128 + 128)
                    nc.tensor.matmul(mm_ps[:, hh * 128:(hh + 1) * 128],
                                     Fk[:, sl], Fv[:, sl],
                                     start=(ti == 0), stop=(ti == 3),
                                     skip_group_check=True)
                    nc.tensor.matmul(mv_ps[:, hh * 128:(hh + 1) * 128],
                                     krb[:, t * 8 + h:t * 8 + h + 1],
                                     Fv[:, sl], start=(ti == 0),
                                     stop=(ti == 3), skip_group_check=True)
            mm = attn.tile([128, 512], BF16, tag=f"mm{j}{g}", name="mm")
            nc.scalar.copy(mm[:], mm_ps[:])
            mv = attn.tile([1, 512], BF16, tag=f"mv{j}{g}", name="mv")
            nc.scalar.copy(mv[:], mv_ps)
            mm_raw[(j, g)] = mm
            mv_raw[(j, g)] = mv

    # -- phase 3 part B: h1 layernorm + relu (DVE/ACT, overlaps fqT) --
    sq3 = late.tile([128, H * 128], BF16)   # write-only square scratch
    w_mu = stat1.tile([8, 4], F32)
    nc.vector.reduce_sum(w_mu[:, 0:1], h1[:], axis=AX.X)
    nc.vector.tensor_scalar_mul(w_mu[:, 0:1], w_mu[:, 0:1], 1.0 / D)
    nc.scalar.activation(sq3[0:8, 0:128], h1[:], AF.Square,
                         accum_out=w_mu[:, 1:2])
    nc.vector.tensor_scalar_mul(w_mu[:, 1:2], w_mu[:, 1:2], 1.0 / D)
    nc.vector.tensor_tensor(w_mu[:, 2:3], w_mu[:, 0:1], w_mu[:, 0:1],
                            op=ALU.mult)
    nc.vector.tensor_tensor(w_mu[:, 2:3], w_mu[:, 1:2], w_mu[:, 2:3],
                            op=ALU.subtract)
    nc.scalar.activation(w_mu[:, 3:4], w_mu[:, 2:3], AF.Sqrt, bias=eps[0:8, :])
    nc.vector.reciprocal(w_mu[:, 3:4], w_mu[:, 3:4])
    h1n = stat1.tile([8, 128], F32)
    nc.vector.tensor_scalar(h1n[:], h1[:], w_mu[:, 0:1], w_mu[:, 3:4],
                            ALU.subtract, ALU.mult)
    nc.vector.tensor_tensor(h1n[:], h1n[:], gbc[:], op=ALU.mult)
    nc.vector.tensor_tensor(h1n[:], h1n[:], bbc[:], op=ALU.add)
    nc.vector.tensor_scalar_max(h1n[:], h1n[:], 0.0)

    # Fq^T tiles (scaled Fq, bf16): 8 transposes (2 head-groups) per PSUM
    # bank tile + 1 wide copy
    fqTs = {}
    for j in range(NTASK):
        for g in range(4):
            tr_ps = ps_trmv.tile([128, 1024], BF16, tag="trmv",
                                 name="tr_ps")
            for hh in range(2):
                h = g * 2 + hh
                for ti in range(4):
                    t = 4 * j + ti
                    sl = slice(t * DIM + h * 128, t * DIM + h * 128 + 128)
                    nc.tensor.transpose(
                        tr_ps[:, hh * 512 + ti * 128:hh * 512 + ti * 128 + 128],
                        Fq[:, sl], identb[:])
            fqT = attn.tile([128, 1024], BF16, tag=f"fqT{j}{g}", name="fqT")
            if g % 2 == 0:
                nc.scalar.copy(fqT[:], tr_ps[:])
            else:
                nc.vector.tensor_copy(fqT[:], tr_ps[:])
            fqTs[(j, g)] = fqT

    # -- phase 3 part C: second predictor layer + softmax --
    nc.tensor.matmul(h1T_ap, h1n[:], identf[:], is_transpose=True,
                     skip_group_check=True)
    h1T = stat1.tile([128, 8], F32)
    nc.scalar.copy(h1T[:], h1T_ap)
    nc.tensor.matmul(lg_ap, h1T[:], wp2T[:], start=True, stop=True,
                     skip_group_check=True)
    lg = stat1.tile([8, 8], F32)
    nc.scalar.copy(lg[:, 0:3], lg_ap)
    nc.vector.tensor_tensor(lg[:, 0:3], lg[:, 0:3], b2bc[:], op=ALU.add)
    # logits are O(1): skip the (mathematically redundant) max-subtraction
    nc.scalar.activation(lg[:, 0:3], lg[:, 0:3], AF.Exp)
    nc.vector.reduce_sum(lg[:, 4:5], lg[:, 0:3], axis=AX.X)
    nc.vector.reciprocal(lg[:, 4:5], lg[:, 4:5])
    nc.vector.tensor_scalar(lg[:, 0:3], lg[:, 0:3], lg[:, 4:5], None, ALU.mult)

    # q_ratio rows: one whole-tile transpose with (j,h,ti)-reordered rows,
    # then contiguous-partition flatten DMAs (partition-strided DMA slices
    # are not supported)
    nc.tensor.matmul(qrT_ap, qrb[:], identb[:], is_transpose=True,
                     skip_group_check=True)
    qrT_sb = stat1.tile([64, 128], BF16)
    nc.vector.tensor_copy(qrT_sb[:], qrT_ap)
    # q_ratio rows flattened pre-AR (DMAs fly during the collective)
    wqrs = {}
    for j in range(NTASK):
        for h in range(H):
            r0 = j * 32 + h * 4
            wqr = attn.tile([1, 512], BF16, tag=f"wqr{h}{j}", name="wqr")
            nc.sync.dma_start(wqr[:], qrT_sb[r0:r0 + 4, :])
            wqrs[(h, j)] = wqr

    # -- phase 3 part D (fc-dependent): decorrelation scale --
    ssq = stat1.tile([128, 8], F32)
    msk = late.tile([128, H * 128], F32)
    nc.vector.tensor_tensor(msk[:], ar[:], mask_nd[:], op=ALU.mult)
    nc.scalar.activation(sq3[:], msk[:], AF.Square, scale=1.0 / TOK_ALL)
    nc.vector.reduce_sum(ssq[:],
                         sq3[:].rearrange("p (h d) -> p h d", h=8),
                         axis=AX.X)
    nc.tensor.matmul(ss_ap, ssq[:], onesf[:], start=True, stop=True,
                     skip_group_check=True)
    dsc = stat1.tile([8, 8], F32)
    nc.scalar.activation(dsc[:, 0:1], ss_ap[0:8, 0:1], AF.Sqrt)
    nc.scalar.activation(dsc[:, 1:2], dsc[:, 0:1], AF.Exp, scale=-5.0 / (D * D))
    # alpha = w0 + w1*dsc ; ww = w2 ; broadcast to 128 partitions
    aw = stat1.tile([8, 2], F32)
    nc.vector.tensor_tensor(aw[:, 0:1], lg[:, 1:2], dsc[:, 1:2], op=ALU.mult)
    nc.vector.tensor_tensor(aw[:, 0:1], aw[:, 0:1], lg[:, 0:1], op=ALU.add)
    nc.vector.tensor_copy(aw[:, 1:2], lg[:, 2:3])
    nc.tensor.matmul(awTa_ap, aw[:, 0:1], identf[:],
                     is_transpose=True, skip_group_check=True)
    nc.tensor.matmul(awTw_ap, aw[:, 1:2], identf[:],
                     is_transpose=True, skip_group_check=True)
    awTa = stat1.tile([1, 8], F32)
    nc.scalar.copy(awTa[:], awTa_ap)
    awTw = stat1.tile([1, 8], F32)
    nc.scalar.copy(awTw[:], awTw_ap)
    abc = stat1.tile([128, 8], F32)
    nc.gpsimd.partition_broadcast(abc[:], awTa[:])
    wbc = stat1.tile([128, 8], F32)
    nc.gpsimd.partition_broadcast(wbc[:], awTw[:])

    # ======== Phase 4b + 5: scaled attention + output projection ========
    with tc.tile_pool(name="ph4", bufs=2) as ph4, \
         tc.tile_pool(name="o1pool", bufs=1) as o1pool:
        o1_tiles = {}
        mm_sb = {}
        mv_sb = {}
        for j in range(NTASK):
            # scale mm/mv by the dynamic per-head weights (batched, 4 heads)
            for g in range(2):
                msb = ph4.tile([128, 512], BF16, tag=f"mmsb{j}{g}",
                               name="mm_sb")
                ab = abc[:, g * 4:(g + 1) * 4].unsqueeze(2) \
                    .broadcast_to([128, 4, 128])
                nc.vector.tensor_tensor(
                    msb[:].rearrange("p (h d) -> p h d", h=4),
                    mm_raw[(j, g)][:].rearrange("p (h d) -> p h d", h=4),
                    ab, op=ALU.mult)
                mm_sb[(j, g)] = msb
                vsb = ph4.tile([1, 512], BF16, tag=f"mvsb{j}{g}",
                               name="mv_sb")
                wb = wbc[0:1, g * 4:(g + 1) * 4].unsqueeze(2) \
                    .broadcast_to([1, 4, 128])
                nc.vector.tensor_tensor(
                    vsb[:].rearrange("p (h d) -> p h d", h=4),
                    mv_raw[(j, g)][:].rearrange("p (h d) -> p h d", h=4),
                    wb, op=ALU.mult)
                mv_sb[(j, g)] = vsb
        for j in range(NTASK):
            for h in range(H):
                g, hh = h // 4, h % 4
                wqr = wqrs[(h, j)]
                opool = ps_o1 if h % 2 == 0 else ps_proj
                o1_ps = opool.tile([128, 512], F32,
                                   tag="o1" if h % 2 == 0 else "proj",
                                   name="o1_ps")
                nc.tensor.matmul(
                    o1_ps[:], mm_sb[(j, g)][:, hh * 128:(hh + 1) * 128],
                    fqTs[(j, h // 2)][:, (h % 2) * 512:(h % 2) * 512 + 512],
                    start=True, stop=False)
                nc.tensor.matmul(o1_ps[:],
                                 mv_sb[(j, g)][:, hh * 128:(hh + 1) * 128],
                                 wqr[:], start=False, stop=True)
                o1 = o1pool.tile([128, 512], BF16, tag=f"o1sb{h}{j}",
                                 name="o1_sb")
                if h % 2 == 0:
                    nc.scalar.copy(o1[:], o1_ps[:])
                else:
                    nc.vector.tensor_copy(o1[:], o1_ps[:])
                o1_tiles[(h, j)] = o1

        # ---- output projection, both tasks ----
        for j in range(NTASK):
            for t in range(4 * j, 4 * j + 4):
                ti = t % 4
                for half in range(2):
                    o = half * 512
                    opool2 = ps_proj if half == 0 else ps_o1
                    op_ps = opool2.tile([128, 512], F32,
                                        tag="proj" if half == 0 else "o1",
                                        name="op_ps")
                    for h in range(H):
                        nc.tensor.matmul(
                            op_ps[:],
                            o1_tiles[(h, j)][:, ti * 128:(ti + 1) * 128],
                            WoT[:, h * DIM + o: h * DIM + o + 512],
                            start=(h == 0), stop=(h == H - 1))
                    ysb = ph4.tile([128, 512], BF16, tag="ysb", name="ysb")
                    nc.vector.tensor_tensor(ysb[:], op_ps[:],
                                            ybias[:, o:o + 512], op=ALU.add)
                    nc.sync.dma_start(y[t * 128:(t + 1) * 128, o:o + 512],
                                      ysb[:])


_BUILT = {}


def _build(n_cores=N_CORES):
    if n_cores in _BUILT:
        return _BUILT[n_cores]
    nc = bacc.Bacc("TRN2", target_bir_lowering=False, debug=False,
                   num_devices=n_cores)
    in_specs = [
        ("xn_q", [T, DIM], BF16), ("xn_k", [T, DIM], BF16),
        ("xn_v", [T, DIM], BF16),
        ("xT_q", [128, NT * DIM], BF16), ("xT_k", [128, NT * DIM], BF16),
        ("xT_v", [128, NT * DIM], BF16),
        ("Wp", [128, 8 * DIM], BF16), ("WoT", [128, 8 * DIM], BF16),
        ("nv2", [2, DIM], BF16), ("bout", [1, DIM], F32),
        ("onesb", [128, 8], BF16), ("onesf", [128, 8], F32),
        ("identb", [128, 128], BF16), ("identf", [8, 8], F32),
        ("mask", [128, 1024], F32),
        ("wp1T", [128, 256], F32), ("wp2T", [128, 3], F32),
        ("b1row", [1, 128], F32),
        ("gbc", [8, 128], F32), ("bbc", [8, 128], F32), ("b2bc", [8, 3], F32),
    ]
    in_aps = [nc.dram_tensor(n, s, d, kind="ExternalInput").ap()
              for n, s, d in in_specs]
    y_ap = nc.dram_tensor("y", [T, DIM], BF16, kind="ExternalOutput").ap()
    with tile.TileContext(nc) as tc:
        attn_kernel(tc, [y_ap], in_aps, n_cores=n_cores)
    nc.compile()
    _BUILT[n_cores] = nc
    return nc


def kernel(q, k, v, ln_g, ln_b, w_in, wp_w1, wp_b1, wp_ln_g, wp_ln_b,
           wp_w2, wp_b2, w_out, b_out):
    q = np.asarray(q, dtype=np.float32)
    k = np.asarray(k, dtype=np.float32)
    v = np.asarray(v, dtype=np.float32)
    ln_g = np.asarray(ln_g, np.float32); ln_b = np.asarray(ln_b, np.float32)
    w_in = np.asarray(w_in, np.float32); w_out = np.asarray(w_out, np.float32)
    b_out = np.asarray(b_out, np.float32)
    wp_w1 = np.asarray(wp_w1, np.float32); wp_b1 = np.asarray(wp_b1, np.float32)
    wp_ln_g = np.asarray(wp_ln_g, np.float32)
    wp_ln_b = np.asarray(wp_ln_b, np.float32)
    wp_w2 = np.asarray(wp_w2, np.float32); wp_b2 = np.asarray(wp_b2, np.float32)

    bf = ml_dtypes.bfloat16

    # host weight prep (folded layernorm)
    W = w_in.T                                     # [DIM, HD]
    Wp = (ln_g[:, None] * W)
    negu = -(ln_g @ W)
    vrow = (ln_b @ W)
    nv2 = np.stack([negu, vrow], axis=0)           # [2, DIM]
    Wp_t = np.ascontiguousarray(
        Wp.reshape(8, 128, DIM).transpose(1, 0, 2)).reshape(128, -1)
    WoT = np.ascontiguousarray(
        w_out.T.reshape(8, 128, DIM).transpose(1, 0, 2)).reshape(128, -1)
    shared = {
        "Wp": Wp_t.astype(bf), "WoT": WoT.astype(bf),
        "nv2": nv2.astype(bf),
        "bout": np.ascontiguousarray(b_out[None, :], np.float32),
        "onesb": np.ones((128, 8), bf),
        "onesf": np.ones((128, 8), np.float32),
        "identb": np.eye(128).astype(bf),
        "identf": np.eye(8, dtype=np.float32),
        "mask": np.ascontiguousarray(
            np.tile((1.0 - np.eye(128)).astype(np.float32), (1, 8))),
        "wp1T": np.ascontiguousarray(wp_w1.T.reshape(2, 128, 128)
                                     .transpose(1, 0, 2)).reshape(128, 256),
        "wp2T": np.ascontiguousarray(wp_w2.T),
        "b1row": np.ascontiguousarray(wp_b1[None, :]),
        "gbc": np.ascontiguousarray(np.tile(wp_ln_g[None, :], (8, 1))),
        "bbc": np.ascontiguousarray(np.tile(wp_ln_b[None, :], (8, 1))),
        "b2bc": np.ascontiguousarray(np.tile(wp_b2[None, :], (8, 1))),
    }

    qf = q.reshape(QB * N, DIM)
    kf = k.reshape(QB * N, DIM)
    vf = v.reshape(QB * N, DIM)
    in_maps = []
    for c in range(N_CORES):
        sl = slice(c * T, (c + 1) * T)
        m = dict(shared)
        for nm, arr in (("q", qf[sl]), ("k", kf[sl]), ("v", vf[sl])):
            ab = arr.astype(bf)
            m[f"xn_{nm}"] = np.ascontiguousarray(ab)
            m[f"xT_{nm}"] = np.ascontiguousarray(
                ab.reshape(NT, 128, 8, 128).transpose(3, 0, 2, 1)
            ).reshape(128, NT * DIM)
        in_maps.append(m)

    nc = _build()
    res = bass_utils.run_bass_kernel_spmd(nc, in_maps,
                                          core_ids=list(range(N_CORES)))
    global LAST_RESULTS
    LAST_RESULTS = res
    out = np.concatenate([np.asarray(r["y"], np.float32)
                          for r in res.results], axis=0)
    return out.reshape(QB, N, DIM)


LAST_RESULTS = None
